# revision 23
# baseline (speedup 1.0000x reference)
"""GATv2 3-layer GNN forward on 8 Trainium2 NeuronCores (Bass/Tile).

Sharding: edges (with self-loops) sorted by dst; core c owns dst nodes
[5000c, 5000(c+1)) so all segment reductions are core-local.

Layer 1 needs NO gathers: xl1[src] = ext5[src] @ WL1 where ext5 (5 cols)
is a host input -- the host pre-gathers ext5[src] per edge and PE applies
the transform per tile with a 5-row stationary. The scatter is linear in
xl, so it runs in 5-dim-per-head space (width 48, WL1 applied after the
scatter via a block-diagonal [40,256] matmul per block).

Layers 2/3 gather xl[src] rows from an AllGathered table (Shared DRAM)
via per-tile indirect DMA (128 rows per instruction -- the HW limit).
Self-loop edges are segregated into each block's last tile and read xl
from the SBUF-resident own-node table instead (no gather).

Per 127-node block: S_T one-hot tiles (+ ea on row 127) come from DRAM;
the scatter one-hot is built ONCE per block by DVE is_equal; exp() output
is written by ScalarE directly into the scatter payload columns.
"""
import sys

for _p in ("/opt/trn_rl_repo",):
    if _p not in sys.path:
        sys.path.insert(0, _p)

import numpy as np

N = 40000
E = 500000
B = 512
NC = 8
NPC = N // NC            # nodes per core
BLK = 122                # real nodes per 128-row block (122 one-hot rows +
                         # 1 ea row + 5 ext5 rows = 128 contraction rows)
NBLK = -(-NPC // BLK)    # blocks per core (41)
PADN = NBLK * 128        # padded node rows per core (5248)
EAROW = 122              # st row holding per-edge ea; 123..127 hold ext5
HEADS = [(8, 32), (8, 16), (1, 8)]   # (H, C) per layer
DIMS = [h * c for h, c in HEADS]     # 256, 128, 8
DS = [40, 128, 8]                    # scatter payload width (no ex cols)
WIDTHS = [48, 136, 9]                # DS + H
GBS = [4, 8, 16]                     # edge tiles per elementwise batch
POOLPAD = 768

_CACHE = {}


def _padrow(n):
    c, nl = np.divmod(n, NPC)
    b, r = np.divmod(nl, BLK)
    return PADN * c + 128 * b + r


def _host_preprocess(x, edge_index, edge_attr, batch):
    src = np.asarray(edge_index[0], np.int64)
    dst = np.asarray(edge_index[1], np.int64)
    ea = np.asarray(edge_attr, np.float32).reshape(-1)

    # self loops, fill_value='mean' of incoming edge_attr
    deg = np.zeros(N, np.float32)
    np.add.at(deg, dst, np.float32(1.0))
    esum = np.zeros(N, np.float32)
    np.add.at(esum, dst, ea)
    loop_attr = np.where(deg > 0, esum / np.maximum(deg, 1.0), 0.0).astype(np.float32)

    # ext5 = [x0, x1, x2, x3, 1] per node
    xf = np.asarray(x, np.float32)
    ext5 = np.concatenate([xf, np.ones((N, 1), np.float32)], axis=1)  # [N,5]

    order = np.argsort(dst, kind="stable")
    src_s, dst_s, ea_s = src[order], dst[order], ea[order]
    src_pad_s = _padrow(src_s).astype(np.int32)

    bounds = np.searchsorted(dst_s, np.arange(0, N + 1, 1))

    # non-self tiles per block (self-loops get their own final tile)
    tiles_pb = []
    for b in range(NBLK):
        mx = 0
        for c in range(NC):
            lo = bounds[min(c * NPC + b * BLK, N)]
            hi = bounds[min(c * NPC + min((b + 1) * BLK, NPC), N)]
            mx = max(mx, hi - lo)
        tiles_pb.append(-(-mx // 128) + 1)   # +1 self tile
    T = sum(tiles_pb)
    MAXNT = max(tiles_pb)

    st_blk = np.zeros((NC, NBLK, 128, MAXNT * 128), np.float32)
    src_all = np.zeros((NC, T, 128), np.int32)
    dst_all = np.full((NC, T, 128), 200.0, np.float32)
    extsrc = np.zeros((NC, 128, T * 5), np.float32)
    t0 = 0
    for b in range(NBLK):
        nt = tiles_pb[b]
        nn = min((b + 1) * BLK, NPC) - b * BLK   # real nodes in block
        for c in range(NC):
            n0 = c * NPC + b * BLK               # first global node id
            lo, hi = bounds[n0], bounds[min(n0 + nn, N)]
            ne = hi - lo
            dl = (dst_s[lo:hi] - n0).astype(np.int64)
            ti = np.arange(ne) // 128
            pi = np.arange(ne) % 128
            st_blk[c, b, dl, ti * 128 + pi] = 1.0
            st_blk[c, b, EAROW, ti * 128 + pi] = ea_s[lo:hi]
            src_all[c, t0 + ti, pi] = src_pad_s[lo:hi]
            dst_all[c, t0 + ti, pi] = dl.astype(np.float32)
            e5 = ext5[src_s[lo:hi]]              # [ne,5]
            for k in range(5):
                st_blk[c, b, 123 + k, ti * 128 + pi] = e5[:, k]
                extsrc[c, pi, (t0 + ti) * 5 + k] = e5[:, k]
            # self tile: slot d = node n0+d, one-hot at (d, d)
            ts = t0 + nt - 1
            dsl = np.arange(nn)
            st_blk[c, b, dsl, (nt - 1) * 128 + dsl] = 1.0
            st_blk[c, b, EAROW, (nt - 1) * 128 + dsl] = loop_attr[n0:n0 + nn]
            dst_all[c, ts, dsl] = dsl.astype(np.float32)
            e5s = ext5[n0:n0 + nn]
            for k in range(5):
                st_blk[c, b, 123 + k, (nt - 1) * 128 + dsl] = e5s[:, k]
                extsrc[c, dsl, ts * 5 + k] = e5s[:, k]
        t0 += nt
    src_sb = np.ascontiguousarray(src_all.transpose(0, 2, 1))
    dst_sb = np.ascontiguousarray(dst_all.transpose(0, 2, 1))

    # pooling metadata
    batch = np.asarray(batch, np.int64)
    gbase = np.array([batch[c * NPC] for c in range(NC)], np.int64)
    batchloc = np.full((NC, 128, NBLK), 200.0, np.float32)
    for c in range(NC):
        bl = batch[c * NPC:(c + 1) * NPC] - gbase[c]
        assert bl.max() < 127, "graph span exceeds 127 per core"
        for b in range(NBLK):
            nn = min((b + 1) * BLK, NPC) - b * BLK
            batchloc[c, :nn, b] = bl[b * BLK: b * BLK + nn]
    g_rows = np.zeros((NC, 128, 1), np.int32)
    for c in range(NC):
        rows = gbase[c] + np.arange(128)
        junk = B + 64 + np.arange(128)
        g_rows[c, :, 0] = np.where(rows < B, rows, junk)
    cnt = np.bincount(batch, minlength=B).astype(np.float32)
    rcnt = (1.0 / np.maximum(cnt, 1.0)).astype(np.float32)

    return dict(tiles_pb=tiles_pb, T=T, MAXNT=MAXNT, st_blk=st_blk, src_sb=src_sb,
                dst_sb=dst_sb, extsrc=extsrc,
                batchloc=batchloc, g_rows=g_rows, rcnt=rcnt)


def _host_weights(inp):
    out = {}
    M = np.zeros((5, 7), np.float32)
    M[0, :4] = np.asarray(inp["w0"], np.float32)[0]
    M[1, 4] = M[2, 5] = M[3, 6] = 1.0
    M[4, :4] = np.asarray(inp["b0"], np.float32)

    for i, (H, C) in enumerate(HEADS, start=1):
        D = H * C
        wl = np.asarray(inp[f"wl{i}"], np.float32)
        wr = np.asarray(inp[f"wr{i}"], np.float32)
        we = np.asarray(inp[f"we{i}"], np.float32)
        att = np.asarray(inp[f"att{i}"], np.float32).reshape(-1)
        absatt = np.maximum(np.abs(att), 1e-8)
        sgn = np.sign(att)
        if i == 1:
            out["WL1A"] = (M @ (wl * absatt[None, :])).astype(np.float32)  # [5,256]
            out["WR1f"] = (M @ (wr * absatt[None, :])).astype(np.float32)
            Mwl = M @ wl                                                   # raw
            WL1B = np.zeros((40, 256), np.float32)
            for h in range(8):
                WL1B[h * 5:(h + 1) * 5, h * 32:(h + 1) * 32] = \
                    Mwl[:, h * 32:(h + 1) * 32]
            out["WL1B"] = WL1B
            out["wl1arep"] = np.tile(out["WL1A"], (1, NBLK))
        else:
            out[f"WL{i}"] = (wl * absatt[None, :]).astype(np.float32)
            out[f"WR{i}"] = (wr * absatt[None, :]).astype(np.float32)
            out[f"attrecip{i}"] = np.tile((1.0 / absatt)[None, :], (128, 1))
        wea = np.tile((we * absatt[None, :]).astype(np.float32), (1, NBLK))
        if i == 1:
            out[f"weaug{i}"] = wea
        else:
            # rows 122..127 of the xr table: ea row + 5 zero ext rows
            out[f"weaug{i}"] = np.concatenate(
                [wea, np.zeros((5, wea.shape[1]), np.float32)], axis=0)
        out[f"sgnB{i}"] = np.tile(sgn[None, :], (128, 1))
        out[f"biasRep{i}"] = np.tile(np.asarray(inp[f"b{i}"], np.float32)[None, :],
                                     (128, 1))
    # layer-2 input transform uses raw wl2 on h1 (att folded above), but the
    # xl tables for gathering need att-folded wl2 -- logits use the gathered
    # rows; messages divide by |att| via attrecip (baseline scheme).
    out["w4rep"] = np.tile(np.asarray(inp["w4"], np.float32)[:, 0][None, :], (128, 1))
    out["b4"] = float(np.asarray(inp["b4"], np.float32)[0])
    return out


def _build_x_inputs(x):
    x = np.asarray(x, np.float32)
    ext = np.concatenate([x, np.ones((N, 1), np.float32)], 1)
    extp = np.zeros((NC * PADN, 5), np.float32)
    extp[_padrow(np.arange(N))] = ext
    xt6_own = np.ascontiguousarray(extp.reshape(NC, PADN, 5).transpose(0, 2, 1))
    return xt6_own


def _build_program(tiles_pb, T, MAXNT):
    import contextlib
    import concourse.bass as bass
    import concourse.bacc as bacc
    import concourse.mybir as mybir
    import concourse.tile as tile

    dt = mybir.dt
    f32 = dt.float32
    bf16 = dt.bfloat16
    i32 = dt.int32
    Alu = mybir.AluOpType
    Act = mybir.ActivationFunctionType
    IOA = bass.IndirectOffsetOnAxis

    nc = bacc.Bacc("TRN2", target_bir_lowering=False, debug=False, num_devices=NC)

    ein = {}
    def EIN(name, shape, d=f32):
        ein[name] = nc.dram_tensor(name, list(shape), d, kind="ExternalInput")
        return ein[name]

    st_blk_d = EIN("st_blk", [NBLK, 128, MAXNT * 128], bf16)
    src_sb_d = EIN("src_sb", [128, T], i32)
    dst_sb_d = EIN("dst_sb", [128, T], bf16)
    extsrc_d = EIN("extsrc", [128, T * 5], bf16)
    xt6_own_d = EIN("xt6_own", [5, PADN])
    wl1arep_d = EIN("wl1arep", [5, NBLK * DIMS[0]], bf16)
    WR1f_d = EIN("WR1f", [5, DIMS[0]])
    WL1B_d = EIN("WL1B", [40, DIMS[0]], bf16)
    WL2_d = EIN("WL2", [DIMS[0], DIMS[1]], bf16)
    WR2_d = EIN("WR2", [DIMS[0], DIMS[1]], bf16)
    WL3_d = EIN("WL3", [DIMS[1], DIMS[2]], bf16)
    WR3_d = EIN("WR3", [DIMS[1], DIMS[2]], bf16)
    weaug_d = [EIN(f"weaug{i}", [1 if i == 1 else 6, NBLK * DIMS[i - 1]], bf16)
               for i in (1, 2, 3)]
    sgnB_d = [EIN(f"sgnB{i}", [128, DIMS[i - 1]], bf16) for i in (1, 2, 3)]
    attrecip_d = [None] + [EIN(f"attrecip{i}", [128, DIMS[i - 1]]) for i in (2, 3)]
    biasRep_d = [EIN(f"biasRep{i}", [128, DIMS[i - 1]]) for i in (1, 2, 3)]
    iota_d = EIN("iota_row", [128, 128], bf16)
    ident_d = EIN("ident", [128, 128], bf16)
    batchloc_d = EIN("batchloc", [128, NBLK], bf16)
    g_rows_d = EIN("g_rows", [128, 1], i32)
    rcnt_d = EIN("rcnt", [128, 4])
    w4rep_d = EIN("w4rep", [128, 8])
    b4_d = EIN("b4v", [128, 1])

    out_d = nc.dram_tensor("out", [B, 1], f32, kind="ExternalOutput")

    stage2 = nc.dram_tensor("stage2", [PADN, DIMS[1]], bf16)
    stage3 = nc.dram_tensor("stage3", [PADN, DIMS[2]], bf16)
    table2 = nc.dram_tensor("table2", [NC * PADN, DIMS[1]], bf16,
                            addr_space="Shared")
    table3 = nc.dram_tensor("table3", [NC * PADN, DIMS[2]], bf16,
                            addr_space="Shared")
    tables = [None, table2, table3]
    pool_full = nc.dram_tensor("pool_full", [POOLPAD, 8], f32)
    pool_red = nc.dram_tensor("pool_red", [B, 8], f32)

    with tile.TileContext(nc) as tc:
        ctx = contextlib.ExitStack()
        with ctx:
            consts = ctx.enter_context(tc.tile_pool(name="consts", bufs=1))
            meta = ctx.enter_context(tc.tile_pool(name="meta", bufs=1))
            xrp = ctx.enter_context(tc.tile_pool(name="xrp", bufs=1))
            stp = ctx.enter_context(tc.tile_pool(name="stp", bufs=2))
            smp = ctx.enter_context(tc.tile_pool(name="smp", bufs=2))
            gp = ctx.enter_context(tc.tile_pool(name="gp", bufs=6))
            sp = ctx.enter_context(tc.tile_pool(name="sp", bufs=3))
            yp = ctx.enter_context(tc.tile_pool(name="yp", bufs=3))
            ep = ctx.enter_context(tc.tile_pool(name="ep", bufs=3))
            pst = ctx.enter_context(tc.tile_pool(name="psum_t", bufs=2, space="PSUM"))
            psb = ctx.enter_context(tc.tile_pool(name="psum_blk", bufs=2, space="PSUM"))
            pse = ctx.enter_context(tc.tile_pool(name="psum_epi", bufs=1, space="PSUM"))
            chp = ctx.enter_context(tc.tile_pool(name="chunk", bufs=2))

            def load_const(dram, shape, d=f32):
                t = consts.tile(list(shape), d, tag=dram.name + "_c")
                nc.sync.dma_start(t[:], dram[:])
                return t
            iota_t = load_const(iota_d, [128, 128], bf16)
            ident_t = load_const(ident_d, [128, 128], bf16)
            WR1f_t = load_const(WR1f_d, [5, DIMS[0]])
            WL1B_t = load_const(WL1B_d, [40, DIMS[0]], bf16)
            WL2_t = [consts.tile([128, DIMS[1]], bf16, tag=f"wl2_{k}",
                                 name=f"wl2_{k}") for k in range(2)]
            WR2_t = [consts.tile([128, DIMS[1]], bf16, tag=f"wr2_{k}",
                                 name=f"wr2_{k}") for k in range(2)]
            for k in range(2):
                nc.sync.dma_start(WL2_t[k][:], WL2_d[k * 128:(k + 1) * 128, :])
                nc.sync.dma_start(WR2_t[k][:], WR2_d[k * 128:(k + 1) * 128, :])
            WL3_t = load_const(WL3_d, [128, DIMS[2]], bf16)
            WR3_t = load_const(WR3_d, [128, DIMS[2]], bf16)
            sgnB_t = [load_const(sgnB_d[i], [128, DIMS[i]], bf16) for i in range(3)]
            attrecip_t = [None] + [load_const(attrecip_d[i - 1], [128, DIMS[i - 1]])
                                   for i in (2, 3)]
            biasRep_t = [load_const(biasRep_d[i], [128, DIMS[i]]) for i in range(3)]
            batchloc_t = load_const(batchloc_d, [128, NBLK], bf16)
            g_rows_t = load_const(g_rows_d, [128, 1], i32)
            rcnt_t = load_const(rcnt_d, [128, 4])
            w4rep_t = load_const(w4rep_d, [128, 8])
            b4_t = load_const(b4_d, [128, 1])
            src_t = meta.tile([128, T], i32)
            nc.sync.dma_start(src_t[:], src_sb_d[:])
            dst_t = meta.tile([128, T], bf16)
            nc.sync.dma_start(dst_t[:], dst_sb_d[:])
            extsrc_t = meta.tile([128, T * 5], bf16)
            nc.sync.dma_start(extsrc_t[:], extsrc_d[:])

            xr_t = [xrp.tile([128, NBLK * DIMS[i]], bf16, tag=f"xr{i}",
                             name=f"xr{i}") for i in range(3)]
            nc.sync.dma_start(xr_t[0][EAROW:EAROW + 1, :], weaug_d[0][:])
            nc.sync.dma_start(xr_t[0][123:128, :], wl1arep_d[:])
            for i in (1, 2):
                nc.sync.dma_start(xr_t[i][EAROW:128, :], weaug_d[i][:])
            # own-node xl tables (self-loop tiles read these instead of gathers)
            xlown_t = [None,
                       xrp.tile([128, NBLK * DIMS[1]], bf16, tag="xlown2",
                                name="xlown2"),
                       xrp.tile([128, NBLK * DIMS[2]], bf16, tag="xlown3",
                                name="xlown3")]

            zero8 = consts.tile([128, 8], f32, tag="zero8")
            nc.gpsimd.memset(zero8[:], 0.0)
            for i in range(POOLPAD // 128):
                nc.sync.dma_start(pool_full[i * 128:(i + 1) * 128, :], zero8[:])

            # ---- preamble: own xr1 (f32 math, bf16 out) ----
            CH = 16
            for ch in range(-(-NBLK // CH)):
                j0, j1 = ch * CH, min((ch + 1) * CH, NBLK)
                xchunk = chp.tile([5, CH * 128], f32, tag="xchunk")
                nc.sync.dma_start(xchunk[:, :(j1 - j0) * 128],
                                  xt6_own_d[:, j0 * 128:j1 * 128])
                for j in range(j1 - j0):
                    b = j0 + j
                    pt = pse.tile([128, DIMS[0]], f32, tag="epi_ps", space="PSUM")
                    nc.tensor.matmul(pt[:], lhsT=xchunk[:, j * 128:(j + 1) * 128],
                                     rhs=WR1f_t[:], start=True, stop=True)
                    D0 = DIMS[0]
                    nc.vector.tensor_copy(xr_t[0][0:BLK, b * D0:(b + 1) * D0],
                                          pt[0:BLK, :])

            # ---- layers ----
            pool_ps = psb.tile([128, 8], f32, tag="pool_ps", space="PSUM", bufs=1)
            for li in range(3):
                H, C = HEADS[li]
                D = DIMS[li]
                D2 = DS[li]
                W = WIDTHS[li]
                GB = GBS[li]
                PSLOT = DIMS[li]
                table = tables[li]
                is_last = li == 2

                if li > 0:
                    nc.gpsimd.collective_compute(
                        "AllGather", Alu.bypass,
                        replica_groups=[list(range(NC))],
                        ins=[(stage2 if li == 1 else stage3).ap().opt()],
                        outs=[table.ap().opt()],
                    )

                t0 = 0
                for b in range(NBLK):
                    nt = tiles_pb[b]
                    pblk = psb.tile([128, W], f32, tag="blk_ps", space="PSUM")
                    sblk = stp.tile([128, MAXNT * 128], bf16, tag="st_blk")
                    nc.scalar.dma_start(sblk[:, :nt * 128],
                                        st_blk_d[b, :, :nt * 128])
                    smat = smp.tile([128, MAXNT * 128], bf16, tag="smat")
                    nc.vector.tensor_tensor(
                        out=smat[:, :nt * 128].rearrange("p (g n) -> p g n", g=nt),
                        in0=dst_t[:, t0:t0 + nt].unsqueeze(2)
                            .to_broadcast([128, nt, 128]),
                        in1=iota_t[:].unsqueeze(1).to_broadcast([128, nt, 128]),
                        op=Alu.is_equal)

                    def do_batch(g0, gs, is_self):
                        ptile = pst.tile([128, GB, PSLOT], f32, tag="t_ps",
                                         space="PSUM")
                        if li > 0 and not is_self:
                            gt = gp.tile([128, GB, D], bf16, tag="g_tile")
                        for i in range(gs):
                            col = g0 + i
                            nc.tensor.matmul(
                                ptile[:, i, 0:D],
                                lhsT=sblk[:, col * 128:(col + 1) * 128],
                                rhs=xr_t[li][:, b * D:(b + 1) * D],
                                start=True, stop=(li == 0))
                            if li == 0:
                                pass
                            elif is_self:
                                nc.tensor.matmul(
                                    ptile[:, i, 0:D], lhsT=ident_t[:],
                                    rhs=xlown_t[li][:, b * D:(b + 1) * D],
                                    start=False, stop=True)
                            else:
                                t = t0 + col
                                nc.gpsimd.indirect_dma_start(
                                    out=gt[:, i, :], out_offset=None,
                                    in_=table[:],
                                    in_offset=IOA(ap=src_t[:, t:t + 1], axis=0))
                                nc.tensor.matmul(
                                    ptile[:, i, 0:D], lhsT=ident_t[:],
                                    rhs=gt[:, i, :], start=False, stop=True)
                        u = sp.tile([128, GBS[0] * DIMS[0]], bf16, tag="u_t")
                        nc.scalar.activation(
                            u[:, :gs * D].rearrange("p (g d) -> p g d", g=gs),
                            ptile[:, 0:gs, 0:D], Act.Prelu, alpha=0.2)
                        v = sp.tile([128, GBS[0] * DIMS[0]], bf16, tag="v_t")
                        nc.vector.tensor_tensor(
                            out=v[:, :gs * D].rearrange("p (g d) -> p g d", g=gs),
                            in0=u[:, :gs * D].rearrange("p (g d) -> p g d", g=gs),
                            in1=sgnB_t[li][:].unsqueeze(1)
                                .to_broadcast([128, gs, D]),
                            op=Alu.mult)
                        lg = sp.tile([128, GBS[2] * 8], f32, tag="lg")
                        nc.vector.tensor_reduce(
                            out=lg[:, :gs * H].rearrange("p (g h) -> p g h", g=gs),
                            in_=v[:, :gs * D].rearrange("p (g h c) -> p g h c",
                                                        g=gs, h=H),
                            axis=mybir.AxisListType.X, op=Alu.add)
                        yt = yp.tile([128, GB, W], bf16, tag="y_tile")
                        nc.scalar.activation(
                            yt[:, 0:gs, D2:W],
                            lg[:, :gs * H].rearrange("p (g h) -> p g h", g=gs),
                            Act.Exp)
                        if li == 0:
                            nc.vector.tensor_tensor(
                                out=yt[:, 0:gs, 0:D2]
                                    .rearrange("p g (h k) -> p g h k", h=H),
                                in0=extsrc_t[:, (t0 + g0) * 5:(t0 + g0 + gs) * 5]
                                    .rearrange("p (g k) -> p g k", g=gs)
                                    .unsqueeze(2).to_broadcast([128, gs, H, 5]),
                                in1=yt[:, 0:gs, D2:W].unsqueeze(3)
                                    .to_broadcast([128, gs, H, 5]),
                                op=Alu.mult)
                        else:
                            if is_self:
                                in0 = xlown_t[li][:, b * D:(b + 1) * D] \
                                    .rearrange("p (h c) -> p h c", h=H) \
                                    .unsqueeze(1)
                            else:
                                in0 = gt[:, 0:gs, :].rearrange(
                                    "p g (h c) -> p g h c", h=H)
                            nc.vector.tensor_tensor(
                                out=yt[:, 0:gs, 0:D2]
                                    .rearrange("p g (h c) -> p g h c", h=H),
                                in0=in0,
                                in1=yt[:, 0:gs, D2:W].unsqueeze(3)
                                    .to_broadcast([128, gs, H, C]),
                                op=Alu.mult)
                        for i in range(gs):
                            col = g0 + i
                            nc.tensor.matmul(
                                pblk[:], lhsT=smat[:, col * 128:(col + 1) * 128],
                                rhs=yt[:, i, 0:W],
                                start=(col == 0), stop=(col == nt - 1))

                    if li == 0:
                        for g0 in range(0, nt, GB):
                            do_batch(g0, min(GB, nt - g0), False)
                    else:
                        for g0 in range(0, nt - 1, GB):
                            do_batch(g0, min(GB, nt - 1 - g0), False)
                        do_batch(nt - 1, 1, True)
                    t0 += nt

                    # ---- block epilogue ----
                    den = sp.tile([128, 8], f32, tag="den")
                    nc.vector.tensor_scalar_add(den[:, 0:H], pblk[:, D2:W], 1e-30)
                    rden = sp.tile([128, 8], f32, tag="rden")
                    nc.vector.reciprocal(rden[:, 0:H], den[:, 0:H])
                    if li == 0:
                        hr5 = ep.tile([128, 40], bf16, tag="hr5")
                        nc.vector.tensor_tensor(
                            out=hr5[:].rearrange("p (h k) -> p h k", h=8),
                            in0=pblk[:, 0:40].rearrange("p (h k) -> p h k", h=8),
                            in1=rden[:, 0:8].unsqueeze(2)
                                .to_broadcast([128, 8, 5]),
                            op=Alu.mult)
                        tp = pse.tile([40, 128], bf16, tag="epi_ps", space="PSUM")
                        nc.tensor.transpose(tp[:], hr5[:], ident_t[:])
                        hT5 = ep.tile([40, 128], bf16, tag="hT5")
                        nc.vector.tensor_copy(hT5[:], tp[:])
                        ph = pse.tile([128, DIMS[0]], f32, tag="epi_ps",
                                      space="PSUM")
                        nc.tensor.matmul(ph[:], lhsT=hT5[:], rhs=WL1B_t[:],
                                         start=True, stop=True)
                        hr = ep.tile([128, DIMS[0]], f32, tag="hr")
                        nc.vector.tensor_tensor(out=hr[:], in0=ph[:],
                                                in1=biasRep_t[0][:], op=Alu.add)
                        h = ep.tile([128, DIMS[0]], bf16, tag="h_blk")
                        nc.scalar.activation(h[:], hr[:], Act.Tanh)
                    else:
                        hr = ep.tile([128, DIMS[0]], f32, tag="hr")
                        nc.vector.tensor_tensor(
                            out=hr[:, 0:D].rearrange("p (h c) -> p h c", h=H),
                            in0=pblk[:, 0:D].rearrange("p (h c) -> p h c", h=H),
                            in1=rden[:, 0:H].unsqueeze(2)
                                .to_broadcast([128, H, C]),
                            op=Alu.mult)
                        nc.vector.tensor_tensor(out=hr[:, 0:D], in0=hr[:, 0:D],
                                                in1=attrecip_t[li][:], op=Alu.mult)
                        nc.vector.tensor_tensor(out=hr[:, 0:D], in0=hr[:, 0:D],
                                                in1=biasRep_t[li][:], op=Alu.add)
                        h = ep.tile([128, DIMS[0]], bf16, tag="h_blk")
                        nc.scalar.activation(h[:, 0:D], hr[:, 0:D], Act.Tanh)

                    if not is_last:
                        D2n = DIMS[li + 1]
                        WLn = [WL2_t[0], WL2_t[1]] if li == 0 else [WL3_t]
                        WRn = [WR2_t[0], WR2_t[1]] if li == 0 else [WR3_t]
                        nk = DIMS[li] // 128
                        hT = []
                        for k in range(nk):
                            tpk = pse.tile([128, 128], bf16, tag="epi_ps",
                                           space="PSUM")
                            nc.tensor.transpose(tpk[:], h[:, k * 128:(k + 1) * 128],
                                                ident_t[:])
                            hTk = ep.tile([128, 128], bf16, tag=f"hT{k}")
                            nc.vector.tensor_copy(hTk[:], tpk[:])
                            hT.append(hTk)
                        pxl = pse.tile([128, D2n], f32, tag="epi_ps", space="PSUM")
                        for k in range(nk):
                            nc.tensor.matmul(pxl[:], lhsT=hT[k][:], rhs=WLn[k][:],
                                             start=(k == 0), stop=(k == nk - 1))
                        xlo = xlown_t[li + 1]
                        nc.vector.tensor_copy(xlo[:, b * D2n:(b + 1) * D2n], pxl[:])
                        nc.sync.dma_start(
                            (stage2 if li == 0 else stage3)[b * 128:(b + 1) * 128, :],
                            xlo[:, b * D2n:(b + 1) * D2n])
                        pxr = pse.tile([128, D2n], f32, tag="epi_ps", space="PSUM")
                        for k in range(nk):
                            nc.tensor.matmul(pxr[:], lhsT=hT[k][:], rhs=WRn[k][:],
                                             start=(k == 0), stop=(k == nk - 1))
                        nc.vector.tensor_copy(
                            xr_t[li + 1][0:BLK, b * D2n:(b + 1) * D2n],
                            pxr[0:BLK, :])
                    else:
                        Sg = smp.tile([128, 128], bf16, tag="sg_tile")
                        nc.vector.tensor_tensor(
                            out=Sg[:],
                            in0=batchloc_t[:, b:b + 1].to_broadcast([128, 128]),
                            in1=iota_t[:], op=Alu.is_equal)
                        nc.tensor.matmul(pool_ps[:], lhsT=Sg[:], rhs=h[:, 0:8],
                                         start=(b == 0), stop=(b == NBLK - 1))

            # ---- pooling + head ----
            pool_sb = ep.tile([128, 8], f32, tag="pool_sb")
            nc.vector.tensor_copy(pool_sb[:], pool_ps[:])
            nc.gpsimd.indirect_dma_start(
                out=pool_full[:], out_offset=IOA(ap=g_rows_t[:, :1], axis=0),
                in_=pool_sb[:], in_offset=None)
            nc.gpsimd.collective_compute(
                "AllReduce", Alu.add, replica_groups=[list(range(NC))],
                ins=[pool_full.ap()[0:B, :].opt()], outs=[pool_red.ap().opt()])
            for i in range(B // 128):
                pt = ep.tile([128, 8], f32, tag="head_in")
                nc.sync.dma_start(pt[:], pool_red[i * 128:(i + 1) * 128, :])
                pw = ep.tile([128, 8], f32, tag="head_w")
                nc.vector.tensor_tensor(out=pw[:], in0=pt[:], in1=w4rep_t[:],
                                        op=Alu.mult)
                hred = ep.tile([128, 1], f32, tag="head_red")
                nc.vector.tensor_reduce(out=hred[:], in_=pw[:],
                                        axis=mybir.AxisListType.X, op=Alu.add)
                nc.vector.tensor_tensor(out=hred[:], in0=hred[:],
                                        in1=rcnt_t[:, i:i + 1], op=Alu.mult)
                nc.vector.tensor_tensor(out=hred[:], in0=hred[:], in1=b4_t[:],
                                        op=Alu.add)
                nc.sync.dma_start(out_d[i * 128:(i + 1) * 128, :], hred[:])

    nc.compile()
    return nc


def _get_program(inputs):
    pre = _host_preprocess(inputs["x"], inputs["edge_index"], inputs["edge_attr"],
                           inputs["batch"])
    key = tuple(pre["tiles_pb"])
    if key not in _CACHE:
        _CACHE[key] = _build_program(pre["tiles_pb"], pre["T"], pre["MAXNT"])
    return _CACHE[key], pre


def _make_in_maps(inputs, pre):
    import ml_dtypes
    bf16 = ml_dtypes.bfloat16
    wts = _host_weights(inputs)
    xt6_own = _build_x_inputs(inputs["x"])
    iota = np.tile(np.arange(128, dtype=np.float32), (128, 1))
    ident = np.eye(128, dtype=np.float32)
    in_maps = []
    for c in range(NC):
        m = dict(
            st_blk=pre["st_blk"][c].astype(bf16), src_sb=pre["src_sb"][c],
            dst_sb=pre["dst_sb"][c].astype(bf16),
            extsrc=pre["extsrc"][c].astype(bf16),
            xt6_own=xt6_own[c],
            wl1arep=wts["wl1arep"].astype(bf16), WR1f=wts["WR1f"],
            WL1B=wts["WL1B"].astype(bf16),
            WL2=wts["WL2"].astype(bf16), WR2=wts["WR2"].astype(bf16),
            WL3=wts["WL3"].astype(bf16), WR3=wts["WR3"].astype(bf16),
            iota_row=iota.astype(bf16), ident=ident.astype(bf16),
            batchloc=pre["batchloc"][c].astype(bf16), g_rows=pre["g_rows"][c],
            rcnt=np.ascontiguousarray(pre["rcnt"].reshape(4, 128).T),
            w4rep=wts["w4rep"], b4v=np.full((128, 1), wts["b4"], np.float32),
        )
        for i in (1, 2, 3):
            m[f"weaug{i}"] = wts[f"weaug{i}"].astype(bf16)
            m[f"sgnB{i}"] = wts[f"sgnB{i}"].astype(bf16)
            m[f"biasRep{i}"] = wts[f"biasRep{i}"]
            if i > 1:
                m[f"attrecip{i}"] = wts[f"attrecip{i}"]
        in_maps.append(m)
    return in_maps


def kernel(**inputs):
    from concourse.bass_utils import run_bass_kernel_spmd
    nc, pre = _get_program(inputs)
    in_maps = _make_in_maps(inputs, pre)
    res = run_bass_kernel_spmd(nc, in_maps, core_ids=list(range(NC)))
    return np.asarray(res.results[0]["out"], np.float32)


# revision 24
# speedup vs baseline: 1.2002x; 1.2002x over previous
"""GATv2 3-layer GNN forward on 8 Trainium2 NeuronCores (Bass/Tile).

Sharding: edges (with self-loops) sorted by dst; core c owns dst nodes
[5000c, 5000(c+1)) so all segment reductions are core-local.

Layer 1 needs NO gathers: xl1[src] = ext5[src] @ WL1 where ext5 (5 cols)
is a host input -- the host pre-gathers ext5[src] per edge and PE applies
the transform per tile with a 5-row stationary. The scatter is linear in
xl, so it runs in 5-dim-per-head space (width 48, WL1 applied after the
scatter via a block-diagonal [40,256] matmul per block).

Layers 2/3 gather xl[src] rows from an AllGathered table (Shared DRAM)
via per-tile indirect DMA (128 rows per instruction -- the HW limit).
Self-loop edges are segregated into each block's last tile and read xl
from the SBUF-resident own-node table instead (no gather).

Per 127-node block: S_T one-hot tiles (+ ea on row 127) come from DRAM;
the scatter one-hot is built ONCE per block by DVE is_equal; exp() output
is written by ScalarE directly into the scatter payload columns.
"""
import sys

for _p in ("/opt/trn_rl_repo",):
    if _p not in sys.path:
        sys.path.insert(0, _p)

import numpy as np

N = 40000
E = 500000
B = 512
NC = 8
NPC = N // NC            # nodes per core
BLK = 122                # real nodes per 128-row block (122 one-hot rows +
                         # 1 ea row + 5 ext5 rows = 128 contraction rows)
NBLK = -(-NPC // BLK)    # blocks per core (41)
PADN = NBLK * 128        # padded node rows per core (5248)
EAROW = 122              # st row holding per-edge ea; 123..127 hold ext5
HEADS = [(8, 32), (8, 16), (1, 8)]   # (H, C) per layer
DIMS = [h * c for h, c in HEADS]     # 256, 128, 8
DS = [40, 128, 8]                    # scatter payload width (no ex cols)
WIDTHS = [48, 136, 9]                # DS + H
GBS = [4, 8, 16]                     # edge tiles per elementwise batch
POOLPAD = 768

_CACHE = {}


def _padrow(n):
    c, nl = np.divmod(n, NPC)
    b, r = np.divmod(nl, BLK)
    return PADN * c + 128 * b + r


def _host_preprocess(x, edge_index, edge_attr, batch):
    src = np.asarray(edge_index[0], np.int64)
    dst = np.asarray(edge_index[1], np.int64)
    ea = np.asarray(edge_attr, np.float32).reshape(-1)

    # self loops, fill_value='mean' of incoming edge_attr
    deg = np.zeros(N, np.float32)
    np.add.at(deg, dst, np.float32(1.0))
    esum = np.zeros(N, np.float32)
    np.add.at(esum, dst, ea)
    loop_attr = np.where(deg > 0, esum / np.maximum(deg, 1.0), 0.0).astype(np.float32)

    # ext5 = [x0, x1, x2, x3, 1] per node
    xf = np.asarray(x, np.float32)
    ext5 = np.concatenate([xf, np.ones((N, 1), np.float32)], axis=1)  # [N,5]

    order = np.argsort(dst, kind="stable")
    src_s, dst_s, ea_s = src[order], dst[order], ea[order]
    src_pad_s = _padrow(src_s).astype(np.int32)

    bounds = np.searchsorted(dst_s, np.arange(0, N + 1, 1))

    # non-self tiles per block (self-loops get their own final tile)
    tiles_pb = []
    for b in range(NBLK):
        mx = 0
        for c in range(NC):
            lo = bounds[min(c * NPC + b * BLK, N)]
            hi = bounds[min(c * NPC + min((b + 1) * BLK, NPC), N)]
            mx = max(mx, hi - lo)
        tiles_pb.append(-(-mx // 128) + 1)   # +1 self tile
    T = sum(tiles_pb)
    MAXNT = max(tiles_pb)

    st_blk = np.zeros((NC, NBLK, 128, MAXNT * 128), np.float32)
    src_all = np.zeros((NC, T, 128), np.int32)
    dst_all = np.full((NC, T, 128), 200.0, np.float32)
    extsrc = np.zeros((NC, 128, T * 5), np.float32)
    t0 = 0
    for b in range(NBLK):
        nt = tiles_pb[b]
        nn = min((b + 1) * BLK, NPC) - b * BLK   # real nodes in block
        for c in range(NC):
            n0 = c * NPC + b * BLK               # first global node id
            lo, hi = bounds[n0], bounds[min(n0 + nn, N)]
            ne = hi - lo
            dl = (dst_s[lo:hi] - n0).astype(np.int64)
            ti = np.arange(ne) // 128
            pi = np.arange(ne) % 128
            st_blk[c, b, dl, ti * 128 + pi] = 1.0
            st_blk[c, b, EAROW, ti * 128 + pi] = ea_s[lo:hi]
            src_all[c, t0 + ti, pi] = src_pad_s[lo:hi]
            dst_all[c, t0 + ti, pi] = dl.astype(np.float32)
            e5 = ext5[src_s[lo:hi]]              # [ne,5]
            for k in range(5):
                st_blk[c, b, 123 + k, ti * 128 + pi] = e5[:, k]
                extsrc[c, pi, (t0 + ti) * 5 + k] = e5[:, k]
            # self tile: slot d = node n0+d, one-hot at (d, d)
            ts = t0 + nt - 1
            dsl = np.arange(nn)
            st_blk[c, b, dsl, (nt - 1) * 128 + dsl] = 1.0
            st_blk[c, b, EAROW, (nt - 1) * 128 + dsl] = loop_attr[n0:n0 + nn]
            dst_all[c, ts, dsl] = dsl.astype(np.float32)
            e5s = ext5[n0:n0 + nn]
            for k in range(5):
                st_blk[c, b, 123 + k, (nt - 1) * 128 + dsl] = e5s[:, k]
                extsrc[c, dsl, ts * 5 + k] = e5s[:, k]
        t0 += nt
    src_sb = np.ascontiguousarray(src_all.transpose(0, 2, 1))
    dst_sb = np.ascontiguousarray(dst_all.transpose(0, 2, 1))

    # pooling metadata
    batch = np.asarray(batch, np.int64)
    gbase = np.array([batch[c * NPC] for c in range(NC)], np.int64)
    batchloc = np.full((NC, 128, NBLK), 200.0, np.float32)
    for c in range(NC):
        bl = batch[c * NPC:(c + 1) * NPC] - gbase[c]
        assert bl.max() < 127, "graph span exceeds 127 per core"
        for b in range(NBLK):
            nn = min((b + 1) * BLK, NPC) - b * BLK
            batchloc[c, :nn, b] = bl[b * BLK: b * BLK + nn]
    g_rows = np.zeros((NC, 128, 1), np.int32)
    for c in range(NC):
        rows = gbase[c] + np.arange(128)
        junk = B + 64 + np.arange(128)
        g_rows[c, :, 0] = np.where(rows < B, rows, junk)
    cnt = np.bincount(batch, minlength=B).astype(np.float32)
    rcnt = (1.0 / np.maximum(cnt, 1.0)).astype(np.float32)

    return dict(tiles_pb=tiles_pb, T=T, MAXNT=MAXNT, st_blk=st_blk, src_sb=src_sb,
                dst_sb=dst_sb, extsrc=extsrc,
                batchloc=batchloc, g_rows=g_rows, rcnt=rcnt)


def _host_weights(inp):
    out = {}
    M = np.zeros((5, 7), np.float32)
    M[0, :4] = np.asarray(inp["w0"], np.float32)[0]
    M[1, 4] = M[2, 5] = M[3, 6] = 1.0
    M[4, :4] = np.asarray(inp["b0"], np.float32)

    for i, (H, C) in enumerate(HEADS, start=1):
        D = H * C
        wl = np.asarray(inp[f"wl{i}"], np.float32)
        wr = np.asarray(inp[f"wr{i}"], np.float32)
        we = np.asarray(inp[f"we{i}"], np.float32)
        att = np.asarray(inp[f"att{i}"], np.float32).reshape(-1)
        absatt = np.maximum(np.abs(att), 1e-8)
        sgn = np.sign(att)
        if i == 1:
            out["WL1A"] = (M @ (wl * absatt[None, :])).astype(np.float32)  # [5,256]
            out["WR1f"] = (M @ (wr * absatt[None, :])).astype(np.float32)
            Mwl = M @ wl                                                   # raw
            WL1B = np.zeros((40, 256), np.float32)
            for h in range(8):
                WL1B[h * 5:(h + 1) * 5, h * 32:(h + 1) * 32] = \
                    Mwl[:, h * 32:(h + 1) * 32]
            out["WL1B"] = WL1B
            out["wl1arep"] = np.tile(out["WL1A"], (1, NBLK))
        else:
            out[f"WL{i}"] = (wl * absatt[None, :]).astype(np.float32)
            out[f"WR{i}"] = (wr * absatt[None, :]).astype(np.float32)
            out[f"attrecip{i}"] = np.tile((1.0 / absatt)[None, :], (128, 1))
        wea = np.tile((we * absatt[None, :]).astype(np.float32), (1, NBLK))
        if i == 1:
            out[f"weaug{i}"] = wea
        else:
            # rows 122..127 of the xr table: ea row + 5 zero ext rows
            out[f"weaug{i}"] = np.concatenate(
                [wea, np.zeros((5, wea.shape[1]), np.float32)], axis=0)
        out[f"sgnB{i}"] = np.tile(sgn[None, :], (128, 1))
        out[f"biasRep{i}"] = np.tile(np.asarray(inp[f"b{i}"], np.float32)[None, :],
                                     (128, 1))
    # layer-2 input transform uses raw wl2 on h1 (att folded above), but the
    # xl tables for gathering need att-folded wl2 -- logits use the gathered
    # rows; messages divide by |att| via attrecip (baseline scheme).
    out["w4rep"] = np.tile(np.asarray(inp["w4"], np.float32)[:, 0][None, :], (128, 1))
    out["b4"] = float(np.asarray(inp["b4"], np.float32)[0])
    return out


def _build_x_inputs(x):
    x = np.asarray(x, np.float32)
    ext = np.concatenate([x, np.ones((N, 1), np.float32)], 1)
    extp = np.zeros((NC * PADN, 5), np.float32)
    extp[_padrow(np.arange(N))] = ext
    xt6_own = np.ascontiguousarray(extp.reshape(NC, PADN, 5).transpose(0, 2, 1))
    return xt6_own


def _build_program(tiles_pb, T, MAXNT):
    import contextlib
    import concourse.bass as bass
    import concourse.bacc as bacc
    import concourse.mybir as mybir
    import concourse.tile as tile

    dt = mybir.dt
    f32 = dt.float32
    bf16 = dt.bfloat16
    i32 = dt.int32
    Alu = mybir.AluOpType
    Act = mybir.ActivationFunctionType
    IOA = bass.IndirectOffsetOnAxis

    nc = bacc.Bacc("TRN2", target_bir_lowering=False, debug=False, num_devices=NC)

    ein = {}
    def EIN(name, shape, d=f32):
        ein[name] = nc.dram_tensor(name, list(shape), d, kind="ExternalInput")
        return ein[name]

    st_blk_d = EIN("st_blk", [NBLK, 128, MAXNT * 128], bf16)
    src_sb_d = EIN("src_sb", [128, T], i32)
    dst_sb_d = EIN("dst_sb", [128, T], bf16)
    extsrc_d = EIN("extsrc", [128, T * 5], bf16)
    xt6_own_d = EIN("xt6_own", [5, PADN])
    wl1arep_d = EIN("wl1arep", [5, NBLK * DIMS[0]], bf16)
    WR1f_d = EIN("WR1f", [5, DIMS[0]])
    WL1B_d = EIN("WL1B", [40, DIMS[0]], bf16)
    WL2_d = EIN("WL2", [DIMS[0], DIMS[1]], bf16)
    WR2_d = EIN("WR2", [DIMS[0], DIMS[1]], bf16)
    WL3_d = EIN("WL3", [DIMS[1], DIMS[2]], bf16)
    WR3_d = EIN("WR3", [DIMS[1], DIMS[2]], bf16)
    weaug_d = [EIN(f"weaug{i}", [1 if i == 1 else 6, NBLK * DIMS[i - 1]], bf16)
               for i in (1, 2, 3)]
    sgnB_d = [EIN(f"sgnB{i}", [128, DIMS[i - 1]], bf16) for i in (1, 2, 3)]
    attrecip_d = [None] + [EIN(f"attrecip{i}", [128, DIMS[i - 1]]) for i in (2, 3)]
    biasRep_d = [EIN(f"biasRep{i}", [128, DIMS[i - 1]]) for i in (1, 2, 3)]
    iota_d = EIN("iota_row", [128, 128], bf16)
    ident_d = EIN("ident", [128, 128], bf16)
    batchloc_d = EIN("batchloc", [128, NBLK], bf16)
    g_rows_d = EIN("g_rows", [128, 1], i32)
    rcnt_d = EIN("rcnt", [128, 4])
    w4rep_d = EIN("w4rep", [128, 8])
    b4_d = EIN("b4v", [128, 1])

    out_d = nc.dram_tensor("out", [B, 1], f32, kind="ExternalOutput")

    stage2 = nc.dram_tensor("stage2", [PADN, DIMS[1]], bf16)
    stage3 = nc.dram_tensor("stage3", [PADN, DIMS[2]], bf16)
    table2 = nc.dram_tensor("table2", [NC * PADN, DIMS[1]], bf16,
                            addr_space="Shared")
    table3 = nc.dram_tensor("table3", [NC * PADN, DIMS[2]], bf16,
                            addr_space="Shared")
    tables = [None, table2, table3]
    pool_full = nc.dram_tensor("pool_full", [POOLPAD, 8], f32)
    pool_red = nc.dram_tensor("pool_red", [B, 8], f32)

    with tile.TileContext(nc) as tc:
        ctx = contextlib.ExitStack()
        with ctx:
            consts = ctx.enter_context(tc.tile_pool(name="consts", bufs=1))
            meta = ctx.enter_context(tc.tile_pool(name="meta", bufs=1))
            xrp = ctx.enter_context(tc.tile_pool(name="xrp", bufs=1))
            stp = ctx.enter_context(tc.tile_pool(name="stp", bufs=2))
            smp = ctx.enter_context(tc.tile_pool(name="smp", bufs=2))
            gp = ctx.enter_context(tc.tile_pool(name="gp", bufs=4))
            sp = ctx.enter_context(tc.tile_pool(name="sp", bufs=3))
            yp = ctx.enter_context(tc.tile_pool(name="yp", bufs=3))
            ep = ctx.enter_context(tc.tile_pool(name="ep", bufs=3))
            pst = ctx.enter_context(tc.tile_pool(name="psum_t", bufs=2, space="PSUM"))
            psb = ctx.enter_context(tc.tile_pool(name="psum_blk", bufs=2, space="PSUM"))
            pse = ctx.enter_context(tc.tile_pool(name="psum_epi", bufs=1, space="PSUM"))
            chp = ctx.enter_context(tc.tile_pool(name="chunk", bufs=2))

            def load_const(dram, shape, d=f32):
                t = consts.tile(list(shape), d, tag=dram.name + "_c")
                nc.sync.dma_start(t[:], dram[:])
                return t
            iota_t = load_const(iota_d, [128, 128], bf16)
            ident_t = load_const(ident_d, [128, 128], bf16)
            WR1f_t = load_const(WR1f_d, [5, DIMS[0]])
            WL1B_t = load_const(WL1B_d, [40, DIMS[0]], bf16)
            WL2_t = [consts.tile([128, DIMS[1]], bf16, tag=f"wl2_{k}",
                                 name=f"wl2_{k}") for k in range(2)]
            WR2_t = [consts.tile([128, DIMS[1]], bf16, tag=f"wr2_{k}",
                                 name=f"wr2_{k}") for k in range(2)]
            for k in range(2):
                nc.sync.dma_start(WL2_t[k][:], WL2_d[k * 128:(k + 1) * 128, :])
                nc.sync.dma_start(WR2_t[k][:], WR2_d[k * 128:(k + 1) * 128, :])
            WL3_t = load_const(WL3_d, [128, DIMS[2]], bf16)
            WR3_t = load_const(WR3_d, [128, DIMS[2]], bf16)
            sgnB_t = [load_const(sgnB_d[i], [128, DIMS[i]], bf16) for i in range(3)]
            attrecip_t = [None] + [load_const(attrecip_d[i - 1], [128, DIMS[i - 1]])
                                   for i in (2, 3)]
            biasRep_t = [load_const(biasRep_d[i], [128, DIMS[i]]) for i in range(3)]
            batchloc_t = load_const(batchloc_d, [128, NBLK], bf16)
            g_rows_t = load_const(g_rows_d, [128, 1], i32)
            rcnt_t = load_const(rcnt_d, [128, 4])
            w4rep_t = load_const(w4rep_d, [128, 8])
            b4_t = load_const(b4_d, [128, 1])
            src_t = meta.tile([128, T], i32)
            nc.sync.dma_start(src_t[:], src_sb_d[:])
            dst_t = meta.tile([128, T], bf16)
            nc.sync.dma_start(dst_t[:], dst_sb_d[:])
            extsrc_t = meta.tile([128, T * 5], bf16)
            nc.sync.dma_start(extsrc_t[:], extsrc_d[:])

            xr_t = [xrp.tile([128, NBLK * DIMS[i]], bf16, tag=f"xr{i}",
                             name=f"xr{i}") for i in range(3)]
            nc.sync.dma_start(xr_t[0][EAROW:EAROW + 1, :], weaug_d[0][:])
            nc.sync.dma_start(xr_t[0][123:128, :], wl1arep_d[:])
            for i in (1, 2):
                nc.sync.dma_start(xr_t[i][EAROW:128, :], weaug_d[i][:])
            # own-node xl tables (self-loop tiles read these instead of gathers)
            xlown_t = [None,
                       xrp.tile([128, NBLK * DIMS[1]], bf16, tag="xlown2",
                                name="xlown2"),
                       xrp.tile([128, NBLK * DIMS[2]], bf16, tag="xlown3",
                                name="xlown3")]

            zero8 = consts.tile([128, 8], f32, tag="zero8")
            nc.gpsimd.memset(zero8[:], 0.0)
            for i in range(POOLPAD // 128):
                nc.sync.dma_start(pool_full[i * 128:(i + 1) * 128, :], zero8[:])

            # ---- preamble: own xr1 (f32 math, bf16 out) ----
            CH = 16
            for ch in range(-(-NBLK // CH)):
                j0, j1 = ch * CH, min((ch + 1) * CH, NBLK)
                xchunk = chp.tile([5, CH * 128], f32, tag="xchunk")
                nc.sync.dma_start(xchunk[:, :(j1 - j0) * 128],
                                  xt6_own_d[:, j0 * 128:j1 * 128])
                for j in range(j1 - j0):
                    b = j0 + j
                    pt = pse.tile([128, DIMS[0]], f32, tag="epi_ps", space="PSUM")
                    nc.tensor.matmul(pt[:], lhsT=xchunk[:, j * 128:(j + 1) * 128],
                                     rhs=WR1f_t[:], start=True, stop=True)
                    D0 = DIMS[0]
                    nc.vector.tensor_copy(xr_t[0][0:BLK, b * D0:(b + 1) * D0],
                                          pt[0:BLK, :])

            # ---- layers ----
            pool_ps = psb.tile([128, 8], f32, tag="pool_ps", space="PSUM", bufs=1)
            for li in range(3):
                H, C = HEADS[li]
                D = DIMS[li]
                D2 = DS[li]
                W = WIDTHS[li]
                GB = GBS[li]
                PSLOT = DIMS[li]
                table = tables[li]
                is_last = li == 2

                if li > 0:
                    nc.gpsimd.collective_compute(
                        "AllGather", Alu.bypass,
                        replica_groups=[list(range(NC))],
                        ins=[(stage2 if li == 1 else stage3).ap().opt()],
                        outs=[table.ap().opt()],
                    )

                t0 = 0
                for b in range(NBLK):
                    nt = tiles_pb[b]
                    pblk = psb.tile([128, W], f32, tag="blk_ps", space="PSUM")
                    sblk = stp.tile([128, MAXNT * 128], bf16, tag="st_blk")
                    nc.scalar.dma_start(sblk[:, :nt * 128],
                                        st_blk_d[b, :, :nt * 128])
                    smat = smp.tile([128, MAXNT * 128], bf16, tag="smat")
                    nc.vector.tensor_tensor(
                        out=smat[:, :nt * 128].rearrange("p (g n) -> p g n", g=nt),
                        in0=dst_t[:, t0:t0 + nt].unsqueeze(2)
                            .to_broadcast([128, nt, 128]),
                        in1=iota_t[:].unsqueeze(1).to_broadcast([128, nt, 128]),
                        op=Alu.is_equal)

                    def do_batch(g0, gs, is_self):
                        ptile = pst.tile([128, GB, PSLOT], f32, tag="t_ps",
                                         space="PSUM")
                        if li > 0 and not is_self:
                            gt = gp.tile([128, GB, D], bf16, tag="g_tile")
                        for i in range(gs):
                            col = g0 + i
                            nc.tensor.matmul(
                                ptile[:, i, 0:D],
                                lhsT=sblk[:, col * 128:(col + 1) * 128],
                                rhs=xr_t[li][:, b * D:(b + 1) * D],
                                start=True, stop=(li == 0))
                            if li == 0:
                                pass
                            elif is_self:
                                nc.tensor.matmul(
                                    ptile[:, i, 0:D], lhsT=ident_t[:],
                                    rhs=xlown_t[li][:, b * D:(b + 1) * D],
                                    start=False, stop=True)
                            else:
                                t = t0 + col
                                nc.gpsimd.indirect_dma_start(
                                    out=gt[:, i, :], out_offset=None,
                                    in_=table[:],
                                    in_offset=IOA(ap=src_t[:, t:t + 1], axis=0))
                                nc.tensor.matmul(
                                    ptile[:, i, 0:D], lhsT=ident_t[:],
                                    rhs=gt[:, i, :], start=False, stop=True)
                        u = sp.tile([128, GBS[0] * DIMS[0]], bf16, tag="u_t")
                        nc.scalar.activation(
                            u[:, :gs * D].rearrange("p (g d) -> p g d", g=gs),
                            ptile[:, 0:gs, 0:D], Act.Prelu, alpha=0.2)
                        v = sp.tile([128, GBS[0] * DIMS[0]], bf16, tag="v_t")
                        nc.vector.tensor_tensor(
                            out=v[:, :gs * D].rearrange("p (g d) -> p g d", g=gs),
                            in0=u[:, :gs * D].rearrange("p (g d) -> p g d", g=gs),
                            in1=sgnB_t[li][:].unsqueeze(1)
                                .to_broadcast([128, gs, D]),
                            op=Alu.mult)
                        lg = sp.tile([128, GBS[2] * 8], f32, tag="lg")
                        nc.vector.tensor_reduce(
                            out=lg[:, :gs * H].rearrange("p (g h) -> p g h", g=gs),
                            in_=v[:, :gs * D].rearrange("p (g h c) -> p g h c",
                                                        g=gs, h=H),
                            axis=mybir.AxisListType.X, op=Alu.add)
                        yt = yp.tile([128, GB, W], bf16, tag="y_tile")
                        nc.scalar.activation(
                            yt[:, 0:gs, D2:W],
                            lg[:, :gs * H].rearrange("p (g h) -> p g h", g=gs),
                            Act.Exp)
                        if li == 0:
                            nc.vector.tensor_tensor(
                                out=yt[:, 0:gs, 0:D2]
                                    .rearrange("p g (h k) -> p g h k", h=H),
                                in0=extsrc_t[:, (t0 + g0) * 5:(t0 + g0 + gs) * 5]
                                    .rearrange("p (g k) -> p g k", g=gs)
                                    .unsqueeze(2).to_broadcast([128, gs, H, 5]),
                                in1=yt[:, 0:gs, D2:W].unsqueeze(3)
                                    .to_broadcast([128, gs, H, 5]),
                                op=Alu.mult)
                        else:
                            if is_self:
                                in0 = xlown_t[li][:, b * D:(b + 1) * D] \
                                    .rearrange("p (h c) -> p h c", h=H) \
                                    .unsqueeze(1)
                            else:
                                in0 = gt[:, 0:gs, :].rearrange(
                                    "p g (h c) -> p g h c", h=H)
                            nc.vector.tensor_tensor(
                                out=yt[:, 0:gs, 0:D2]
                                    .rearrange("p g (h c) -> p g h c", h=H),
                                in0=in0,
                                in1=yt[:, 0:gs, D2:W].unsqueeze(3)
                                    .to_broadcast([128, gs, H, C]),
                                op=Alu.mult)
                        for i in range(gs):
                            col = g0 + i
                            nc.tensor.matmul(
                                pblk[:], lhsT=smat[:, col * 128:(col + 1) * 128],
                                rhs=yt[:, i, 0:W],
                                start=(col == 0), stop=(col == nt - 1))

                    if li == 0:
                        for g0 in range(0, nt, GB):
                            do_batch(g0, min(GB, nt - g0), False)
                    else:
                        for g0 in range(0, nt - 1, GB):
                            do_batch(g0, min(GB, nt - 1 - g0), False)
                        do_batch(nt - 1, 1, True)
                    t0 += nt

                    # ---- block epilogue ----
                    den = sp.tile([128, 8], f32, tag="den")
                    nc.vector.tensor_scalar_add(den[:, 0:H], pblk[:, D2:W], 1e-30)
                    rden = sp.tile([128, 8], f32, tag="rden")
                    nc.vector.reciprocal(rden[:, 0:H], den[:, 0:H])
                    if li == 0:
                        hr5 = ep.tile([128, 40], bf16, tag="hr5")
                        nc.vector.tensor_tensor(
                            out=hr5[:].rearrange("p (h k) -> p h k", h=8),
                            in0=pblk[:, 0:40].rearrange("p (h k) -> p h k", h=8),
                            in1=rden[:, 0:8].unsqueeze(2)
                                .to_broadcast([128, 8, 5]),
                            op=Alu.mult)
                        tp = pse.tile([40, 128], bf16, tag="epi_ps", space="PSUM")
                        nc.tensor.transpose(tp[:], hr5[:], ident_t[:])
                        hT5 = ep.tile([40, 128], bf16, tag="hT5")
                        nc.vector.tensor_copy(hT5[:], tp[:])
                        ph = pse.tile([128, DIMS[0]], f32, tag="epi_ps",
                                      space="PSUM")
                        nc.tensor.matmul(ph[:], lhsT=hT5[:], rhs=WL1B_t[:],
                                         start=True, stop=True)
                        hr = ep.tile([128, DIMS[0]], f32, tag="hr")
                        nc.vector.tensor_tensor(out=hr[:], in0=ph[:],
                                                in1=biasRep_t[0][:], op=Alu.add)
                        h = ep.tile([128, DIMS[0]], bf16, tag="h_blk")
                        nc.scalar.activation(h[:], hr[:], Act.Tanh)
                    else:
                        hr = ep.tile([128, DIMS[0]], f32, tag="hr")
                        nc.vector.tensor_tensor(
                            out=hr[:, 0:D].rearrange("p (h c) -> p h c", h=H),
                            in0=pblk[:, 0:D].rearrange("p (h c) -> p h c", h=H),
                            in1=rden[:, 0:H].unsqueeze(2)
                                .to_broadcast([128, H, C]),
                            op=Alu.mult)
                        nc.vector.tensor_tensor(out=hr[:, 0:D], in0=hr[:, 0:D],
                                                in1=attrecip_t[li][:], op=Alu.mult)
                        nc.vector.tensor_tensor(out=hr[:, 0:D], in0=hr[:, 0:D],
                                                in1=biasRep_t[li][:], op=Alu.add)
                        h = ep.tile([128, DIMS[0]], bf16, tag="h_blk")
                        nc.scalar.activation(h[:, 0:D], hr[:, 0:D], Act.Tanh)

                    if not is_last:
                        D2n = DIMS[li + 1]
                        WLn = [WL2_t[0], WL2_t[1]] if li == 0 else [WL3_t]
                        WRn = [WR2_t[0], WR2_t[1]] if li == 0 else [WR3_t]
                        nk = DIMS[li] // 128
                        hT = []
                        for k in range(nk):
                            tpk = pse.tile([128, 128], bf16, tag="epi_ps",
                                           space="PSUM")
                            nc.tensor.transpose(tpk[:], h[:, k * 128:(k + 1) * 128],
                                                ident_t[:])
                            hTk = ep.tile([128, 128], bf16, tag=f"hT{k}")
                            nc.vector.tensor_copy(hTk[:], tpk[:])
                            hT.append(hTk)
                        pxl = pse.tile([128, D2n], f32, tag="epi_ps", space="PSUM")
                        for k in range(nk):
                            nc.tensor.matmul(pxl[:], lhsT=hT[k][:], rhs=WLn[k][:],
                                             start=(k == 0), stop=(k == nk - 1))
                        xlo = xlown_t[li + 1]
                        nc.vector.tensor_copy(xlo[:, b * D2n:(b + 1) * D2n], pxl[:])
                        nc.sync.dma_start(
                            (stage2 if li == 0 else stage3)[b * 128:(b + 1) * 128, :],
                            xlo[:, b * D2n:(b + 1) * D2n])
                        pxr = pse.tile([128, D2n], f32, tag="epi_ps", space="PSUM")
                        for k in range(nk):
                            nc.tensor.matmul(pxr[:], lhsT=hT[k][:], rhs=WRn[k][:],
                                             start=(k == 0), stop=(k == nk - 1))
                        nc.vector.tensor_copy(
                            xr_t[li + 1][0:BLK, b * D2n:(b + 1) * D2n],
                            pxr[0:BLK, :])
                    else:
                        Sg = smp.tile([128, 128], bf16, tag="sg_tile")
                        nc.vector.tensor_tensor(
                            out=Sg[:],
                            in0=batchloc_t[:, b:b + 1].to_broadcast([128, 128]),
                            in1=iota_t[:], op=Alu.is_equal)
                        nc.tensor.matmul(pool_ps[:], lhsT=Sg[:], rhs=h[:, 0:8],
                                         start=(b == 0), stop=(b == NBLK - 1))

            # ---- pooling + head ----
            pool_sb = ep.tile([128, 8], f32, tag="pool_sb")
            nc.vector.tensor_copy(pool_sb[:], pool_ps[:])
            nc.gpsimd.indirect_dma_start(
                out=pool_full[:], out_offset=IOA(ap=g_rows_t[:, :1], axis=0),
                in_=pool_sb[:], in_offset=None)
            nc.gpsimd.collective_compute(
                "AllReduce", Alu.add, replica_groups=[list(range(NC))],
                ins=[pool_full.ap()[0:B, :].opt()], outs=[pool_red.ap().opt()])
            for i in range(B // 128):
                pt = ep.tile([128, 8], f32, tag="head_in")
                nc.sync.dma_start(pt[:], pool_red[i * 128:(i + 1) * 128, :])
                pw = ep.tile([128, 8], f32, tag="head_w")
                nc.vector.tensor_tensor(out=pw[:], in0=pt[:], in1=w4rep_t[:],
                                        op=Alu.mult)
                hred = ep.tile([128, 1], f32, tag="head_red")
                nc.vector.tensor_reduce(out=hred[:], in_=pw[:],
                                        axis=mybir.AxisListType.X, op=Alu.add)
                nc.vector.tensor_tensor(out=hred[:], in0=hred[:],
                                        in1=rcnt_t[:, i:i + 1], op=Alu.mult)
                nc.vector.tensor_tensor(out=hred[:], in0=hred[:], in1=b4_t[:],
                                        op=Alu.add)
                nc.sync.dma_start(out_d[i * 128:(i + 1) * 128, :], hred[:])

    nc.compile()
    return nc


def _get_program(inputs):
    pre = _host_preprocess(inputs["x"], inputs["edge_index"], inputs["edge_attr"],
                           inputs["batch"])
    key = tuple(pre["tiles_pb"])
    if key not in _CACHE:
        _CACHE[key] = _build_program(pre["tiles_pb"], pre["T"], pre["MAXNT"])
    return _CACHE[key], pre


def _make_in_maps(inputs, pre):
    import ml_dtypes
    bf16 = ml_dtypes.bfloat16
    wts = _host_weights(inputs)
    xt6_own = _build_x_inputs(inputs["x"])
    iota = np.tile(np.arange(128, dtype=np.float32), (128, 1))
    ident = np.eye(128, dtype=np.float32)
    in_maps = []
    for c in range(NC):
        m = dict(
            st_blk=pre["st_blk"][c].astype(bf16), src_sb=pre["src_sb"][c],
            dst_sb=pre["dst_sb"][c].astype(bf16),
            extsrc=pre["extsrc"][c].astype(bf16),
            xt6_own=xt6_own[c],
            wl1arep=wts["wl1arep"].astype(bf16), WR1f=wts["WR1f"],
            WL1B=wts["WL1B"].astype(bf16),
            WL2=wts["WL2"].astype(bf16), WR2=wts["WR2"].astype(bf16),
            WL3=wts["WL3"].astype(bf16), WR3=wts["WR3"].astype(bf16),
            iota_row=iota.astype(bf16), ident=ident.astype(bf16),
            batchloc=pre["batchloc"][c].astype(bf16), g_rows=pre["g_rows"][c],
            rcnt=np.ascontiguousarray(pre["rcnt"].reshape(4, 128).T),
            w4rep=wts["w4rep"], b4v=np.full((128, 1), wts["b4"], np.float32),
        )
        for i in (1, 2, 3):
            m[f"weaug{i}"] = wts[f"weaug{i}"].astype(bf16)
            m[f"sgnB{i}"] = wts[f"sgnB{i}"].astype(bf16)
            m[f"biasRep{i}"] = wts[f"biasRep{i}"]
            if i > 1:
                m[f"attrecip{i}"] = wts[f"attrecip{i}"]
        in_maps.append(m)
    return in_maps


def kernel(**inputs):
    from concourse.bass_utils import run_bass_kernel_spmd
    nc, pre = _get_program(inputs)
    in_maps = _make_in_maps(inputs, pre)
    res = run_bass_kernel_spmd(nc, in_maps, core_ids=list(range(NC)))
    return np.asarray(res.results[0]["out"], np.float32)


# revision 25
# speedup vs baseline: 1.2396x; 1.0328x over previous
"""GATv2 3-layer GNN forward on 8 Trainium2 NeuronCores (Bass/Tile).

Sharding: edges (with self-loops) sorted by dst; core c owns dst nodes
[5000c, 5000(c+1)) so all segment reductions are core-local.

Layer 1 needs NO gathers: xl1[src] = ext5[src] @ WL1 where ext5 (5 cols)
is a host input -- the host pre-gathers ext5[src] per edge and PE applies
the transform per tile with a 5-row stationary. The scatter is linear in
xl, so it runs in 5-dim-per-head space (width 48, WL1 applied after the
scatter via a block-diagonal [40,256] matmul per block).

Layers 2/3 gather xl[src] rows from an AllGathered table (Shared DRAM)
via per-tile indirect DMA (128 rows per instruction -- the HW limit).
Self-loop edges are segregated into each block's last tile and read xl
from the SBUF-resident own-node table instead (no gather).

Per 127-node block: S_T one-hot tiles (+ ea on row 127) come from DRAM;
the scatter one-hot is built ONCE per block by DVE is_equal; exp() output
is written by ScalarE directly into the scatter payload columns.
"""
import sys

for _p in ("/opt/trn_rl_repo",):
    if _p not in sys.path:
        sys.path.insert(0, _p)

import numpy as np

N = 40000
E = 500000
B = 512
NC = 8
NPC = N // NC            # nodes per core
BLK = 122                # real nodes per 128-row block (122 one-hot rows +
                         # 1 ea row + 5 ext5 rows = 128 contraction rows)
NBLK = -(-NPC // BLK)    # blocks per core (41)
PADN = NBLK * 128        # padded node rows per core (5248)
EAROW = 122              # st row holding per-edge ea; 123..127 hold ext5
HEADS = [(8, 32), (8, 16), (1, 8)]   # (H, C) per layer
DIMS = [h * c for h, c in HEADS]     # 256, 128, 8
DS = [40, 128, 8]                    # scatter payload width (no ex cols)
WIDTHS = [48, 136, 9]                # DS + H
GBS = [4, 8, 8]                     # edge tiles per elementwise batch
POOLPAD = 768

_CACHE = {}


def _padrow(n):
    c, nl = np.divmod(n, NPC)
    b, r = np.divmod(nl, BLK)
    return PADN * c + 128 * b + r


def _host_preprocess(x, edge_index, edge_attr, batch):
    src = np.asarray(edge_index[0], np.int64)
    dst = np.asarray(edge_index[1], np.int64)
    ea = np.asarray(edge_attr, np.float32).reshape(-1)

    # self loops, fill_value='mean' of incoming edge_attr
    deg = np.zeros(N, np.float32)
    np.add.at(deg, dst, np.float32(1.0))
    esum = np.zeros(N, np.float32)
    np.add.at(esum, dst, ea)
    loop_attr = np.where(deg > 0, esum / np.maximum(deg, 1.0), 0.0).astype(np.float32)

    # ext5 = [x0, x1, x2, x3, 1] per node
    xf = np.asarray(x, np.float32)
    ext5 = np.concatenate([xf, np.ones((N, 1), np.float32)], axis=1)  # [N,5]

    order = np.argsort(dst, kind="stable")
    src_s, dst_s, ea_s = src[order], dst[order], ea[order]
    src_pad_s = _padrow(src_s).astype(np.int32)

    bounds = np.searchsorted(dst_s, np.arange(0, N + 1, 1))

    # non-self tiles per block (self-loops get their own final tile)
    tiles_pb = []
    for b in range(NBLK):
        mx = 0
        for c in range(NC):
            lo = bounds[min(c * NPC + b * BLK, N)]
            hi = bounds[min(c * NPC + min((b + 1) * BLK, NPC), N)]
            mx = max(mx, hi - lo)
        tiles_pb.append(-(-mx // 128) + 1)   # +1 self tile
    T = sum(tiles_pb)
    MAXNT = max(tiles_pb)

    st_blk = np.zeros((NC, NBLK, 128, MAXNT * 128), np.float32)
    src_all = np.zeros((NC, T, 128), np.int32)
    dst_all = np.full((NC, T, 128), 200.0, np.float32)
    extsrc = np.zeros((NC, 128, T * 5), np.float32)
    t0 = 0
    for b in range(NBLK):
        nt = tiles_pb[b]
        nn = min((b + 1) * BLK, NPC) - b * BLK   # real nodes in block
        for c in range(NC):
            n0 = c * NPC + b * BLK               # first global node id
            lo, hi = bounds[n0], bounds[min(n0 + nn, N)]
            ne = hi - lo
            dl = (dst_s[lo:hi] - n0).astype(np.int64)
            ti = np.arange(ne) // 128
            pi = np.arange(ne) % 128
            st_blk[c, b, dl, ti * 128 + pi] = 1.0
            st_blk[c, b, EAROW, ti * 128 + pi] = ea_s[lo:hi]
            src_all[c, t0 + ti, pi] = src_pad_s[lo:hi]
            dst_all[c, t0 + ti, pi] = dl.astype(np.float32)
            e5 = ext5[src_s[lo:hi]]              # [ne,5]
            for k in range(5):
                st_blk[c, b, 123 + k, ti * 128 + pi] = e5[:, k]
                extsrc[c, pi, (t0 + ti) * 5 + k] = e5[:, k]
            # self tile: slot d = node n0+d, one-hot at (d, d)
            ts = t0 + nt - 1
            dsl = np.arange(nn)
            st_blk[c, b, dsl, (nt - 1) * 128 + dsl] = 1.0
            st_blk[c, b, EAROW, (nt - 1) * 128 + dsl] = loop_attr[n0:n0 + nn]
            dst_all[c, ts, dsl] = dsl.astype(np.float32)
            e5s = ext5[n0:n0 + nn]
            for k in range(5):
                st_blk[c, b, 123 + k, (nt - 1) * 128 + dsl] = e5s[:, k]
                extsrc[c, dsl, ts * 5 + k] = e5s[:, k]
        t0 += nt
    src_sb = np.ascontiguousarray(src_all.transpose(0, 2, 1))
    dst_sb = np.ascontiguousarray(dst_all.transpose(0, 2, 1))

    # pooling metadata
    batch = np.asarray(batch, np.int64)
    gbase = np.array([batch[c * NPC] for c in range(NC)], np.int64)
    batchloc = np.full((NC, 128, NBLK), 200.0, np.float32)
    for c in range(NC):
        bl = batch[c * NPC:(c + 1) * NPC] - gbase[c]
        assert bl.max() < 127, "graph span exceeds 127 per core"
        for b in range(NBLK):
            nn = min((b + 1) * BLK, NPC) - b * BLK
            batchloc[c, :nn, b] = bl[b * BLK: b * BLK + nn]
    g_rows = np.zeros((NC, 128, 1), np.int32)
    for c in range(NC):
        rows = gbase[c] + np.arange(128)
        junk = B + 64 + np.arange(128)
        g_rows[c, :, 0] = np.where(rows < B, rows, junk)
    cnt = np.bincount(batch, minlength=B).astype(np.float32)
    rcnt = (1.0 / np.maximum(cnt, 1.0)).astype(np.float32)

    return dict(tiles_pb=tiles_pb, T=T, MAXNT=MAXNT, st_blk=st_blk, src_sb=src_sb,
                dst_sb=dst_sb, extsrc=extsrc,
                batchloc=batchloc, g_rows=g_rows, rcnt=rcnt)


def _host_weights(inp):
    out = {}
    M = np.zeros((5, 7), np.float32)
    M[0, :4] = np.asarray(inp["w0"], np.float32)[0]
    M[1, 4] = M[2, 5] = M[3, 6] = 1.0
    M[4, :4] = np.asarray(inp["b0"], np.float32)

    for i, (H, C) in enumerate(HEADS, start=1):
        D = H * C
        wl = np.asarray(inp[f"wl{i}"], np.float32)
        wr = np.asarray(inp[f"wr{i}"], np.float32)
        we = np.asarray(inp[f"we{i}"], np.float32)
        att = np.asarray(inp[f"att{i}"], np.float32).reshape(-1)
        absatt = np.maximum(np.abs(att), 1e-8)
        sgn = np.sign(att)
        if i == 1:
            out["WL1A"] = (M @ (wl * absatt[None, :])).astype(np.float32)  # [5,256]
            out["WR1f"] = (M @ (wr * absatt[None, :])).astype(np.float32)
            Mwl = M @ wl                                                   # raw
            WL1B = np.zeros((41, 256), np.float32)
            for h in range(8):
                WL1B[h * 5:(h + 1) * 5, h * 32:(h + 1) * 32] = \
                    Mwl[:, h * 32:(h + 1) * 32]
            WL1B[40, :] = np.asarray(inp["b1"], np.float32)
            out["WL1B"] = WL1B
            out["wl1arep"] = np.tile(out["WL1A"], (1, NBLK))
        else:
            out[f"WL{i}"] = (wl * absatt[None, :]).astype(np.float32)
            out[f"WR{i}"] = (wr * absatt[None, :]).astype(np.float32)
            out[f"attrecip{i}"] = np.tile((1.0 / absatt)[None, :], (128, 1))
        wea = np.tile((we * absatt[None, :]).astype(np.float32), (1, NBLK))
        if i == 1:
            out[f"weaug{i}"] = wea
        else:
            # rows 122..127 of the xr table: ea row + 5 zero ext rows
            out[f"weaug{i}"] = np.concatenate(
                [wea, np.zeros((5, wea.shape[1]), np.float32)], axis=0)
        out[f"sgnB{i}"] = np.tile(sgn[None, :], (128, GBS[i - 1]))
        out[f"biasRep{i}"] = np.tile(np.asarray(inp[f"b{i}"], np.float32)[None, :],
                                     (128, 1))
    # layer-2 input transform uses raw wl2 on h1 (att folded above), but the
    # xl tables for gathering need att-folded wl2 -- logits use the gathered
    # rows; messages divide by |att| via attrecip (baseline scheme).
    out["w4rep"] = np.tile(np.asarray(inp["w4"], np.float32)[:, 0][None, :], (128, 1))
    out["b4"] = float(np.asarray(inp["b4"], np.float32)[0])
    return out


def _build_x_inputs(x):
    x = np.asarray(x, np.float32)
    ext = np.concatenate([x, np.ones((N, 1), np.float32)], 1)
    extp = np.zeros((NC * PADN, 5), np.float32)
    extp[_padrow(np.arange(N))] = ext
    xt6_own = np.ascontiguousarray(extp.reshape(NC, PADN, 5).transpose(0, 2, 1))
    return xt6_own


def _build_program(tiles_pb, T, MAXNT):
    import contextlib
    import concourse.bass as bass
    import concourse.bacc as bacc
    import concourse.mybir as mybir
    import concourse.tile as tile

    dt = mybir.dt
    f32 = dt.float32
    bf16 = dt.bfloat16
    i32 = dt.int32
    Alu = mybir.AluOpType
    Act = mybir.ActivationFunctionType
    IOA = bass.IndirectOffsetOnAxis

    nc = bacc.Bacc("TRN2", target_bir_lowering=False, debug=False, num_devices=NC)

    ein = {}
    def EIN(name, shape, d=f32):
        ein[name] = nc.dram_tensor(name, list(shape), d, kind="ExternalInput")
        return ein[name]

    st_blk_d = EIN("st_blk", [NBLK, 128, MAXNT * 128], bf16)
    src_sb_d = EIN("src_sb", [128, T], i32)
    dst_sb_d = EIN("dst_sb", [128, T], bf16)
    extsrc_d = EIN("extsrc", [128, T * 5], bf16)
    xt6_own_d = EIN("xt6_own", [5, PADN])
    wl1arep_d = EIN("wl1arep", [5, NBLK * DIMS[0]], bf16)
    WR1f_d = EIN("WR1f", [5, DIMS[0]])
    WL1B_d = EIN("WL1B", [41, DIMS[0]], bf16)
    WL2_d = EIN("WL2", [DIMS[0], DIMS[1]], bf16)
    WR2_d = EIN("WR2", [DIMS[0], DIMS[1]], bf16)
    WL3_d = EIN("WL3", [DIMS[1], DIMS[2]], bf16)
    WR3_d = EIN("WR3", [DIMS[1], DIMS[2]], bf16)
    weaug_d = [EIN(f"weaug{i}", [1 if i == 1 else 6, NBLK * DIMS[i - 1]], bf16)
               for i in (1, 2, 3)]
    sgnB_d = [EIN(f"sgnB{i}", [128, GBS[i - 1] * DIMS[i - 1]], bf16)
              for i in (1, 2, 3)]
    attrecip_d = [None] + [EIN(f"attrecip{i}", [128, DIMS[i - 1]]) for i in (2, 3)]
    biasRep_d = [EIN(f"biasRep{i}", [128, DIMS[i - 1]]) for i in (1, 2, 3)]
    iota_d = EIN("iota_row", [128, 128], bf16)
    ident_d = EIN("ident", [128, 128], bf16)
    batchloc_d = EIN("batchloc", [128, NBLK], bf16)
    g_rows_d = EIN("g_rows", [128, 1], i32)
    rcnt_d = EIN("rcnt", [128, 4])
    w4rep_d = EIN("w4rep", [128, 8])
    b4_d = EIN("b4v", [128, 1])

    out_d = nc.dram_tensor("out", [B, 1], f32, kind="ExternalOutput")

    stage2 = nc.dram_tensor("stage2", [PADN, DIMS[1]], bf16)
    stage3 = nc.dram_tensor("stage3", [PADN, DIMS[2]], bf16)
    table2 = nc.dram_tensor("table2", [NC * PADN, DIMS[1]], bf16,
                            addr_space="Shared")
    table3 = nc.dram_tensor("table3", [NC * PADN, DIMS[2]], bf16,
                            addr_space="Shared")
    tables = [None, table2, table3]
    pool_full = nc.dram_tensor("pool_full", [POOLPAD, 8], f32)
    pool_red = nc.dram_tensor("pool_red", [B, 8], f32)

    with tile.TileContext(nc) as tc:
        ctx = contextlib.ExitStack()
        with ctx:
            consts = ctx.enter_context(tc.tile_pool(name="consts", bufs=1))
            meta = ctx.enter_context(tc.tile_pool(name="meta", bufs=1))
            xrp = ctx.enter_context(tc.tile_pool(name="xrp", bufs=1))
            stp = ctx.enter_context(tc.tile_pool(name="stp", bufs=2))
            smp = ctx.enter_context(tc.tile_pool(name="smp", bufs=2))
            gp = ctx.enter_context(tc.tile_pool(name="gp", bufs=4))
            sp = ctx.enter_context(tc.tile_pool(name="sp", bufs=3))
            yp = ctx.enter_context(tc.tile_pool(name="yp", bufs=3))
            ep = ctx.enter_context(tc.tile_pool(name="ep", bufs=3))
            pst = ctx.enter_context(tc.tile_pool(name="psum_t", bufs=2, space="PSUM"))
            psb = ctx.enter_context(tc.tile_pool(name="psum_blk", bufs=2, space="PSUM"))
            pse = ctx.enter_context(tc.tile_pool(name="psum_epi", bufs=1, space="PSUM"))
            chp = ctx.enter_context(tc.tile_pool(name="chunk", bufs=2))

            def load_const(dram, shape, d=f32):
                t = consts.tile(list(shape), d, tag=dram.name + "_c")
                nc.sync.dma_start(t[:], dram[:])
                return t
            iota_t = load_const(iota_d, [128, 128], bf16)
            ident_t = load_const(ident_d, [128, 128], bf16)
            WR1f_t = load_const(WR1f_d, [5, DIMS[0]])
            WL1B_t = load_const(WL1B_d, [41, DIMS[0]], bf16)
            WL2_t = [consts.tile([128, DIMS[1]], bf16, tag=f"wl2_{k}",
                                 name=f"wl2_{k}") for k in range(2)]
            WR2_t = [consts.tile([128, DIMS[1]], bf16, tag=f"wr2_{k}",
                                 name=f"wr2_{k}") for k in range(2)]
            for k in range(2):
                nc.sync.dma_start(WL2_t[k][:], WL2_d[k * 128:(k + 1) * 128, :])
                nc.sync.dma_start(WR2_t[k][:], WR2_d[k * 128:(k + 1) * 128, :])
            WL3_t = load_const(WL3_d, [128, DIMS[2]], bf16)
            WR3_t = load_const(WR3_d, [128, DIMS[2]], bf16)
            sgnB_t = [load_const(sgnB_d[i], [128, GBS[i] * DIMS[i]], bf16)
                      for i in range(3)]
            attrecip_t = [None] + [load_const(attrecip_d[i - 1], [128, DIMS[i - 1]])
                                   for i in (2, 3)]
            biasRep_t = [load_const(biasRep_d[i], [128, DIMS[i]]) for i in range(3)]
            batchloc_t = load_const(batchloc_d, [128, NBLK], bf16)
            g_rows_t = load_const(g_rows_d, [128, 1], i32)
            rcnt_t = load_const(rcnt_d, [128, 4])
            w4rep_t = load_const(w4rep_d, [128, 8])
            b4_t = load_const(b4_d, [128, 1])
            src_t = meta.tile([128, T], i32)
            nc.sync.dma_start(src_t[:], src_sb_d[:])
            dst_t = meta.tile([128, T], bf16)
            nc.sync.dma_start(dst_t[:], dst_sb_d[:])
            extsrc_t = meta.tile([128, T * 5], bf16)
            nc.sync.dma_start(extsrc_t[:], extsrc_d[:])

            xr_t = [xrp.tile([128, NBLK * DIMS[i]], bf16, tag=f"xr{i}",
                             name=f"xr{i}") for i in range(3)]
            nc.sync.dma_start(xr_t[0][EAROW:EAROW + 1, :], weaug_d[0][:])
            nc.sync.dma_start(xr_t[0][123:128, :], wl1arep_d[:])
            for i in (1, 2):
                nc.sync.dma_start(xr_t[i][EAROW:128, :], weaug_d[i][:])
            # own-node xl tables (self-loop tiles read these instead of gathers)
            xlown_t = [None,
                       xrp.tile([128, NBLK * DIMS[1]], bf16, tag="xlown2",
                                name="xlown2"),
                       xrp.tile([128, NBLK * DIMS[2]], bf16, tag="xlown3",
                                name="xlown3")]

            zero8 = consts.tile([128, 8], f32, tag="zero8")
            nc.gpsimd.memset(zero8[:], 0.0)
            for i in range(POOLPAD // 128):
                nc.sync.dma_start(pool_full[i * 128:(i + 1) * 128, :], zero8[:])

            # ---- preamble: own xr1 (f32 math, bf16 out) ----
            CH = 16
            for ch in range(-(-NBLK // CH)):
                j0, j1 = ch * CH, min((ch + 1) * CH, NBLK)
                xchunk = chp.tile([5, CH * 128], f32, tag="xchunk")
                nc.sync.dma_start(xchunk[:, :(j1 - j0) * 128],
                                  xt6_own_d[:, j0 * 128:j1 * 128])
                for j in range(j1 - j0):
                    b = j0 + j
                    pt = pse.tile([128, DIMS[0]], f32, tag="epi_ps", space="PSUM")
                    nc.tensor.matmul(pt[:], lhsT=xchunk[:, j * 128:(j + 1) * 128],
                                     rhs=WR1f_t[:], start=True, stop=True)
                    D0 = DIMS[0]
                    nc.scalar.copy(xr_t[0][0:BLK, b * D0:(b + 1) * D0],
                                   pt[0:BLK, :])

            # ---- layers ----
            pool_ps = psb.tile([128, 8], f32, tag="pool_ps", space="PSUM", bufs=1)
            for li in range(3):
                H, C = HEADS[li]
                D = DIMS[li]
                D2 = DS[li]
                W = WIDTHS[li]
                GB = GBS[li]
                PSLOT = DIMS[li]
                table = tables[li]
                is_last = li == 2

                if li > 0:
                    nc.gpsimd.collective_compute(
                        "AllGather", Alu.bypass,
                        replica_groups=[list(range(NC))],
                        ins=[(stage2 if li == 1 else stage3).ap().opt()],
                        outs=[table.ap().opt()],
                    )

                t0 = 0
                for b in range(NBLK):
                    nt = tiles_pb[b]
                    pblk = psb.tile([128, W], f32, tag="blk_ps", space="PSUM")
                    sblk = stp.tile([128, MAXNT * 128], bf16, tag="st_blk")
                    nc.scalar.dma_start(sblk[:, :nt * 128],
                                        st_blk_d[b, :, :nt * 128])
                    smat = smp.tile([128, MAXNT * 128], bf16, tag="smat")
                    nc.vector.tensor_tensor(
                        out=smat[:, :nt * 128].rearrange("p (g n) -> p g n", g=nt),
                        in0=dst_t[:, t0:t0 + nt].unsqueeze(2)
                            .to_broadcast([128, nt, 128]),
                        in1=iota_t[:].unsqueeze(1).to_broadcast([128, nt, 128]),
                        op=Alu.is_equal)

                    def do_batch(g0, gs, is_self):
                        ptile = pst.tile([128, GB, PSLOT], f32, tag="t_ps",
                                         space="PSUM")
                        if li > 0 and not is_self:
                            gt = gp.tile([128, GB, D], bf16, tag="g_tile")
                        for i in range(gs):
                            col = g0 + i
                            nc.tensor.matmul(
                                ptile[:, i, 0:D],
                                lhsT=sblk[:, col * 128:(col + 1) * 128],
                                rhs=xr_t[li][:, b * D:(b + 1) * D],
                                start=True, stop=(li == 0))
                            if li == 0:
                                pass
                            elif is_self:
                                nc.tensor.matmul(
                                    ptile[:, i, 0:D], lhsT=ident_t[:],
                                    rhs=xlown_t[li][:, b * D:(b + 1) * D],
                                    start=False, stop=True)
                            else:
                                t = t0 + col
                                nc.gpsimd.indirect_dma_start(
                                    out=gt[:, i, :], out_offset=None,
                                    in_=table[:],
                                    in_offset=IOA(ap=src_t[:, t:t + 1], axis=0))
                                nc.tensor.matmul(
                                    ptile[:, i, 0:D], lhsT=ident_t[:],
                                    rhs=gt[:, i, :], start=False, stop=True)
                        u = sp.tile([128, GBS[0] * DIMS[0]], bf16, tag="u_t")
                        nc.scalar.activation(
                            u[:, :gs * D].rearrange("p (g d) -> p g d", g=gs),
                            ptile[:, 0:gs, 0:D], Act.Prelu, alpha=0.2)
                        v = sp.tile([128, GBS[0] * DIMS[0]], bf16, tag="v_t")
                        nc.vector.tensor_tensor(
                            out=v[:, :gs * D], in0=u[:, :gs * D],
                            in1=sgnB_t[li][:, :gs * D], op=Alu.mult)
                        lg = sp.tile([128, GBS[2] * 8], f32, tag="lg")
                        nc.vector.tensor_reduce(
                            out=lg[:, :gs * H].rearrange("p (g h) -> p g h", g=gs),
                            in_=v[:, :gs * D].rearrange("p (g h c) -> p g h c",
                                                        g=gs, h=H),
                            axis=mybir.AxisListType.X, op=Alu.add)
                        yt = yp.tile([128, GB, W], bf16, tag="y_tile")
                        nc.scalar.activation(
                            yt[:, 0:gs, D2:W],
                            lg[:, :gs * H].rearrange("p (g h) -> p g h", g=gs),
                            Act.Exp)
                        if li == 0:
                            nc.vector.tensor_tensor(
                                out=yt[:, 0:gs, 0:D2]
                                    .rearrange("p g (h k) -> p g h k", h=H),
                                in0=extsrc_t[:, (t0 + g0) * 5:(t0 + g0 + gs) * 5]
                                    .rearrange("p (g k) -> p g k", g=gs)
                                    .unsqueeze(2).to_broadcast([128, gs, H, 5]),
                                in1=yt[:, 0:gs, D2:W].unsqueeze(3)
                                    .to_broadcast([128, gs, H, 5]),
                                op=Alu.mult)
                        else:
                            if is_self:
                                in0 = xlown_t[li][:, b * D:(b + 1) * D] \
                                    .rearrange("p (h c) -> p h c", h=H) \
                                    .unsqueeze(1)
                            else:
                                in0 = gt[:, 0:gs, :].rearrange(
                                    "p g (h c) -> p g h c", h=H)
                            nc.vector.tensor_tensor(
                                out=yt[:, 0:gs, 0:D2]
                                    .rearrange("p g (h c) -> p g h c", h=H),
                                in0=in0,
                                in1=yt[:, 0:gs, D2:W].unsqueeze(3)
                                    .to_broadcast([128, gs, H, C]),
                                op=Alu.mult)
                        for i in range(gs):
                            col = g0 + i
                            nc.tensor.matmul(
                                pblk[:], lhsT=smat[:, col * 128:(col + 1) * 128],
                                rhs=yt[:, i, 0:W],
                                start=(col == 0), stop=(col == nt - 1))

                    if li == 0:
                        for g0 in range(0, nt, GB):
                            do_batch(g0, min(GB, nt - g0), False)
                    else:
                        for g0 in range(0, nt - 1, GB):
                            do_batch(g0, min(GB, nt - 1 - g0), False)
                        do_batch(nt - 1, 1, True)
                    t0 += nt

                    # ---- block epilogue ----
                    den = sp.tile([128, 8], f32, tag="den")
                    nc.vector.tensor_scalar_add(den[:, 0:H], pblk[:, D2:W], 1e-30)
                    rden = sp.tile([128, 8], f32, tag="rden")
                    nc.vector.reciprocal(rden[:, 0:H], den[:, 0:H])
                    if li == 0:
                        hr5 = ep.tile([128, 41], bf16, tag="hr5")
                        nc.vector.memset(hr5[:, 40:41], 1.0)
                        nc.vector.tensor_tensor(
                            out=hr5[:, 0:40].rearrange("p (h k) -> p h k", h=8),
                            in0=pblk[:, 0:40].rearrange("p (h k) -> p h k", h=8),
                            in1=rden[:, 0:8].unsqueeze(2)
                                .to_broadcast([128, 8, 5]),
                            op=Alu.mult)
                        tp = pse.tile([41, 128], bf16, tag="epi_ps", space="PSUM")
                        nc.tensor.transpose(tp[:], hr5[:], ident_t[:])
                        hT5 = ep.tile([41, 128], bf16, tag="hT5")
                        nc.scalar.copy(hT5[:], tp[:])
                        ph = pse.tile([128, DIMS[0]], f32, tag="epi_ps",
                                      space="PSUM")
                        nc.tensor.matmul(ph[:], lhsT=hT5[:], rhs=WL1B_t[:],
                                         start=True, stop=True)
                        h = ep.tile([128, DIMS[0]], bf16, tag="h_blk")
                        nc.scalar.activation(h[:], ph[:], Act.Tanh)
                    else:
                        hr = ep.tile([128, DIMS[0]], f32, tag="hr")
                        nc.vector.tensor_tensor(
                            out=hr[:, 0:D].rearrange("p (h c) -> p h c", h=H),
                            in0=pblk[:, 0:D].rearrange("p (h c) -> p h c", h=H),
                            in1=rden[:, 0:H].unsqueeze(2)
                                .to_broadcast([128, H, C]),
                            op=Alu.mult)
                        nc.vector.tensor_tensor(out=hr[:, 0:D], in0=hr[:, 0:D],
                                                in1=attrecip_t[li][:], op=Alu.mult)
                        nc.vector.tensor_tensor(out=hr[:, 0:D], in0=hr[:, 0:D],
                                                in1=biasRep_t[li][:], op=Alu.add)
                        h = ep.tile([128, DIMS[0]], bf16, tag="h_blk")
                        nc.scalar.activation(h[:, 0:D], hr[:, 0:D], Act.Tanh)

                    if not is_last:
                        D2n = DIMS[li + 1]
                        WLn = [WL2_t[0], WL2_t[1]] if li == 0 else [WL3_t]
                        WRn = [WR2_t[0], WR2_t[1]] if li == 0 else [WR3_t]
                        nk = DIMS[li] // 128
                        hT = []
                        for k in range(nk):
                            tpk = pse.tile([128, 128], bf16, tag="epi_ps",
                                           space="PSUM")
                            nc.tensor.transpose(tpk[:], h[:, k * 128:(k + 1) * 128],
                                                ident_t[:])
                            hTk = ep.tile([128, 128], bf16, tag=f"hT{k}")
                            nc.scalar.copy(hTk[:], tpk[:])
                            hT.append(hTk)
                        pxl = pse.tile([128, D2n], f32, tag="epi_ps", space="PSUM")
                        for k in range(nk):
                            nc.tensor.matmul(pxl[:], lhsT=hT[k][:], rhs=WLn[k][:],
                                             start=(k == 0), stop=(k == nk - 1))
                        xlo = xlown_t[li + 1]
                        nc.scalar.copy(xlo[:, b * D2n:(b + 1) * D2n], pxl[:])
                        nc.sync.dma_start(
                            (stage2 if li == 0 else stage3)[b * 128:(b + 1) * 128, :],
                            xlo[:, b * D2n:(b + 1) * D2n])
                        pxr = pse.tile([128, D2n], f32, tag="epi_ps", space="PSUM")
                        for k in range(nk):
                            nc.tensor.matmul(pxr[:], lhsT=hT[k][:], rhs=WRn[k][:],
                                             start=(k == 0), stop=(k == nk - 1))
                        nc.scalar.copy(
                            xr_t[li + 1][0:BLK, b * D2n:(b + 1) * D2n],
                            pxr[0:BLK, :])
                    else:
                        Sg = smp.tile([128, 128], bf16, tag="sg_tile")
                        nc.vector.tensor_tensor(
                            out=Sg[:],
                            in0=batchloc_t[:, b:b + 1].to_broadcast([128, 128]),
                            in1=iota_t[:], op=Alu.is_equal)
                        nc.tensor.matmul(pool_ps[:], lhsT=Sg[:], rhs=h[:, 0:8],
                                         start=(b == 0), stop=(b == NBLK - 1))

            # ---- pooling + head ----
            pool_sb = ep.tile([128, 8], f32, tag="pool_sb")
            nc.vector.tensor_copy(pool_sb[:], pool_ps[:])
            nc.gpsimd.indirect_dma_start(
                out=pool_full[:], out_offset=IOA(ap=g_rows_t[:, :1], axis=0),
                in_=pool_sb[:], in_offset=None)
            nc.gpsimd.collective_compute(
                "AllReduce", Alu.add, replica_groups=[list(range(NC))],
                ins=[pool_full.ap()[0:B, :].opt()], outs=[pool_red.ap().opt()])
            for i in range(B // 128):
                pt = ep.tile([128, 8], f32, tag="head_in")
                nc.sync.dma_start(pt[:], pool_red[i * 128:(i + 1) * 128, :])
                pw = ep.tile([128, 8], f32, tag="head_w")
                nc.vector.tensor_tensor(out=pw[:], in0=pt[:], in1=w4rep_t[:],
                                        op=Alu.mult)
                hred = ep.tile([128, 1], f32, tag="head_red")
                nc.vector.tensor_reduce(out=hred[:], in_=pw[:],
                                        axis=mybir.AxisListType.X, op=Alu.add)
                nc.vector.tensor_tensor(out=hred[:], in0=hred[:],
                                        in1=rcnt_t[:, i:i + 1], op=Alu.mult)
                nc.vector.tensor_tensor(out=hred[:], in0=hred[:], in1=b4_t[:],
                                        op=Alu.add)
                nc.sync.dma_start(out_d[i * 128:(i + 1) * 128, :], hred[:])

    nc.compile()
    return nc


def _get_program(inputs):
    pre = _host_preprocess(inputs["x"], inputs["edge_index"], inputs["edge_attr"],
                           inputs["batch"])
    key = tuple(pre["tiles_pb"])
    if key not in _CACHE:
        _CACHE[key] = _build_program(pre["tiles_pb"], pre["T"], pre["MAXNT"])
    return _CACHE[key], pre


def _make_in_maps(inputs, pre):
    import ml_dtypes
    bf16 = ml_dtypes.bfloat16
    wts = _host_weights(inputs)
    xt6_own = _build_x_inputs(inputs["x"])
    iota = np.tile(np.arange(128, dtype=np.float32), (128, 1))
    ident = np.eye(128, dtype=np.float32)
    in_maps = []
    for c in range(NC):
        m = dict(
            st_blk=pre["st_blk"][c].astype(bf16), src_sb=pre["src_sb"][c],
            dst_sb=pre["dst_sb"][c].astype(bf16),
            extsrc=pre["extsrc"][c].astype(bf16),
            xt6_own=xt6_own[c],
            wl1arep=wts["wl1arep"].astype(bf16), WR1f=wts["WR1f"],
            WL1B=wts["WL1B"].astype(bf16),
            WL2=wts["WL2"].astype(bf16), WR2=wts["WR2"].astype(bf16),
            WL3=wts["WL3"].astype(bf16), WR3=wts["WR3"].astype(bf16),
            iota_row=iota.astype(bf16), ident=ident.astype(bf16),
            batchloc=pre["batchloc"][c].astype(bf16), g_rows=pre["g_rows"][c],
            rcnt=np.ascontiguousarray(pre["rcnt"].reshape(4, 128).T),
            w4rep=wts["w4rep"], b4v=np.full((128, 1), wts["b4"], np.float32),
        )
        for i in (1, 2, 3):
            m[f"weaug{i}"] = wts[f"weaug{i}"].astype(bf16)
            m[f"sgnB{i}"] = wts[f"sgnB{i}"].astype(bf16)
            m[f"biasRep{i}"] = wts[f"biasRep{i}"]
            if i > 1:
                m[f"attrecip{i}"] = wts[f"attrecip{i}"]
        in_maps.append(m)
    return in_maps


def kernel(**inputs):
    from concourse.bass_utils import run_bass_kernel_spmd
    nc, pre = _get_program(inputs)
    in_maps = _make_in_maps(inputs, pre)
    res = run_bass_kernel_spmd(nc, in_maps, core_ids=list(range(NC)))
    return np.asarray(res.results[0]["out"], np.float32)


# revision 28
# speedup vs baseline: 1.2652x; 1.0206x over previous
"""GATv2 3-layer GNN forward on 8 Trainium2 NeuronCores (Bass/Tile).

Sharding: edges (with self-loops) sorted by dst; core c owns dst nodes
[5000c, 5000(c+1)) so all segment reductions are core-local.

Layer 1 needs NO gathers: xl1[src] = ext5[src] @ WL1 where ext5 (5 cols)
is a host input -- the host pre-gathers ext5[src] per edge and PE applies
the transform per tile with a 5-row stationary. The scatter is linear in
xl, so it runs in 5-dim-per-head space (width 48, WL1 applied after the
scatter via a block-diagonal [40,256] matmul per block).

Layers 2/3 gather xl[src] rows from an AllGathered table (Shared DRAM)
via per-tile indirect DMA (128 rows per instruction -- the HW limit).
Self-loop edges are segregated into each block's last tile and read xl
from the SBUF-resident own-node table instead (no gather).

Per 127-node block: S_T one-hot tiles (+ ea on row 127) come from DRAM;
the scatter one-hot is built ONCE per block by DVE is_equal; exp() output
is written by ScalarE directly into the scatter payload columns.
"""
import sys

for _p in ("/opt/trn_rl_repo",):
    if _p not in sys.path:
        sys.path.insert(0, _p)

import numpy as np

N = 40000
E = 500000
B = 512
NC = 8
NPC = N // NC            # nodes per core
BLK = 122                # real nodes per 128-row block (122 one-hot rows +
                         # 1 ea row + 5 ext5 rows = 128 contraction rows)
NBLK = -(-NPC // BLK)    # blocks per core (41)
PADN = NBLK * 128        # padded node rows per core (5248)
EAROW = 122              # st row holding per-edge ea; 123..127 hold ext5
HEADS = [(8, 32), (8, 16), (1, 8)]   # (H, C) per layer
DIMS = [h * c for h, c in HEADS]     # 256, 128, 8
DS = [40, 128, 8]                    # scatter payload width (no ex cols)
WIDTHS = [48, 136, 9]                # DS + H
GBS = [4, 8, 8]                     # edge tiles per elementwise batch
POOLPAD = 768
KCH = 4                  # AllGather chunks (table layout is chunk-major)
CHB = [NBLK - (KCH - 1) * (NBLK // KCH)] + [NBLK // KCH] * (KCH - 1)
PB = np.cumsum([0] + CHB)[:KCH]          # first block of each chunk
CH_OF_B = np.repeat(np.arange(KCH), CHB)  # block -> chunk
CHBASE = np.cumsum([0] + [NC * 128 * c for c in CHB])[:KCH]

_CACHE = {}


def _padrow(n):
    c, nl = np.divmod(n, NPC)
    b, r = np.divmod(nl, BLK)
    k = CH_OF_B[b]
    return (CHBASE[k] + c * 128 * np.asarray(CHB)[k]
            + (b - PB[k]) * 128 + r)


def _host_preprocess(x, edge_index, edge_attr, batch):
    src = np.asarray(edge_index[0], np.int64)
    dst = np.asarray(edge_index[1], np.int64)
    ea = np.asarray(edge_attr, np.float32).reshape(-1)

    # self loops, fill_value='mean' of incoming edge_attr
    deg = np.zeros(N, np.float32)
    np.add.at(deg, dst, np.float32(1.0))
    esum = np.zeros(N, np.float32)
    np.add.at(esum, dst, ea)
    loop_attr = np.where(deg > 0, esum / np.maximum(deg, 1.0), 0.0).astype(np.float32)

    # ext5 = [x0, x1, x2, x3, 1] per node
    xf = np.asarray(x, np.float32)
    ext5 = np.concatenate([xf, np.ones((N, 1), np.float32)], axis=1)  # [N,5]

    order = np.argsort(dst, kind="stable")
    src_s, dst_s, ea_s = src[order], dst[order], ea[order]
    src_pad_s = _padrow(src_s).astype(np.int32)

    bounds = np.searchsorted(dst_s, np.arange(0, N + 1, 1))

    # non-self tiles per block (self-loops get their own final tile)
    tiles_pb = []
    for b in range(NBLK):
        mx = 0
        for c in range(NC):
            lo = bounds[min(c * NPC + b * BLK, N)]
            hi = bounds[min(c * NPC + min((b + 1) * BLK, NPC), N)]
            mx = max(mx, hi - lo)
        tiles_pb.append(-(-mx // 128) + 1)   # +1 self tile
    T = sum(tiles_pb)
    MAXNT = max(tiles_pb)

    st_blk = np.zeros((NC, NBLK, 128, MAXNT * 128), np.float32)
    src_all = np.zeros((NC, T, 128), np.int32)
    dst_all = np.full((NC, T, 128), 200.0, np.float32)
    extsrc = np.zeros((NC, 128, T * 5), np.float32)
    t0 = 0
    for b in range(NBLK):
        nt = tiles_pb[b]
        nn = min((b + 1) * BLK, NPC) - b * BLK   # real nodes in block
        for c in range(NC):
            n0 = c * NPC + b * BLK               # first global node id
            lo, hi = bounds[n0], bounds[min(n0 + nn, N)]
            ne = hi - lo
            dl = (dst_s[lo:hi] - n0).astype(np.int64)
            ti = np.arange(ne) // 128
            pi = np.arange(ne) % 128
            st_blk[c, b, dl, ti * 128 + pi] = 1.0
            st_blk[c, b, EAROW, ti * 128 + pi] = ea_s[lo:hi]
            src_all[c, t0 + ti, pi] = src_pad_s[lo:hi]
            dst_all[c, t0 + ti, pi] = dl.astype(np.float32)
            e5 = ext5[src_s[lo:hi]]              # [ne,5]
            for k in range(5):
                st_blk[c, b, 123 + k, ti * 128 + pi] = e5[:, k]
                extsrc[c, pi, (t0 + ti) * 5 + k] = e5[:, k]
            # self tile: slot d = node n0+d, one-hot at (d, d)
            ts = t0 + nt - 1
            dsl = np.arange(nn)
            st_blk[c, b, dsl, (nt - 1) * 128 + dsl] = 1.0
            st_blk[c, b, EAROW, (nt - 1) * 128 + dsl] = loop_attr[n0:n0 + nn]
            dst_all[c, ts, dsl] = dsl.astype(np.float32)
            e5s = ext5[n0:n0 + nn]
            for k in range(5):
                st_blk[c, b, 123 + k, (nt - 1) * 128 + dsl] = e5s[:, k]
                extsrc[c, dsl, ts * 5 + k] = e5s[:, k]
        t0 += nt
    src_sb = np.ascontiguousarray(src_all.transpose(0, 2, 1))
    dst_sb = np.ascontiguousarray(dst_all.transpose(0, 2, 1))

    # pooling metadata
    batch = np.asarray(batch, np.int64)
    gbase = np.array([batch[c * NPC] for c in range(NC)], np.int64)
    batchloc = np.full((NC, 128, NBLK), 200.0, np.float32)
    for c in range(NC):
        bl = batch[c * NPC:(c + 1) * NPC] - gbase[c]
        assert bl.max() < 127, "graph span exceeds 127 per core"
        for b in range(NBLK):
            nn = min((b + 1) * BLK, NPC) - b * BLK
            batchloc[c, :nn, b] = bl[b * BLK: b * BLK + nn]
    g_rows = np.zeros((NC, 128, 1), np.int32)
    for c in range(NC):
        rows = gbase[c] + np.arange(128)
        junk = B + 64 + np.arange(128)
        g_rows[c, :, 0] = np.where(rows < B, rows, junk)
    cnt = np.bincount(batch, minlength=B).astype(np.float32)
    rcnt = (1.0 / np.maximum(cnt, 1.0)).astype(np.float32)

    return dict(tiles_pb=tiles_pb, T=T, MAXNT=MAXNT, st_blk=st_blk, src_sb=src_sb,
                dst_sb=dst_sb, extsrc=extsrc,
                batchloc=batchloc, g_rows=g_rows, rcnt=rcnt)


def _host_weights(inp):
    out = {}
    M = np.zeros((5, 7), np.float32)
    M[0, :4] = np.asarray(inp["w0"], np.float32)[0]
    M[1, 4] = M[2, 5] = M[3, 6] = 1.0
    M[4, :4] = np.asarray(inp["b0"], np.float32)

    for i, (H, C) in enumerate(HEADS, start=1):
        D = H * C
        wl = np.asarray(inp[f"wl{i}"], np.float32)
        wr = np.asarray(inp[f"wr{i}"], np.float32)
        we = np.asarray(inp[f"we{i}"], np.float32)
        att = np.asarray(inp[f"att{i}"], np.float32).reshape(-1)
        absatt = np.maximum(np.abs(att), 1e-8)
        sgn = np.sign(att)
        if i == 1:
            out["WL1A"] = (M @ (wl * absatt[None, :])).astype(np.float32)  # [5,256]
            out["WR1f"] = (M @ (wr * absatt[None, :])).astype(np.float32)
            Mwl = M @ wl                                                   # raw
            WL1B = np.zeros((41, 256), np.float32)
            for h in range(8):
                WL1B[h * 5:(h + 1) * 5, h * 32:(h + 1) * 32] = \
                    Mwl[:, h * 32:(h + 1) * 32]
            WL1B[40, :] = np.asarray(inp["b1"], np.float32)
            out["WL1B"] = WL1B
            out["wl1arep"] = np.tile(out["WL1A"], (1, NBLK))
        else:
            out[f"WL{i}"] = (wl * absatt[None, :]).astype(np.float32)
            out[f"WR{i}"] = (wr * absatt[None, :]).astype(np.float32)
            out[f"WLR{i}"] = np.concatenate(
                [out[f"WL{i}"], out[f"WR{i}"]], axis=1)
            out[f"attrecip{i}"] = np.tile((1.0 / absatt)[None, :], (128, 1))
        wea = np.tile((we * absatt[None, :]).astype(np.float32), (1, NBLK))
        if i == 1:
            out[f"weaug{i}"] = wea
        else:
            # rows 122..127 of the xr table: ea row + 5 zero ext rows
            out[f"weaug{i}"] = np.concatenate(
                [wea, np.zeros((5, wea.shape[1]), np.float32)], axis=0)
        out[f"sgnB{i}"] = np.tile(sgn[None, :], (128, GBS[i - 1]))
        out[f"biasRep{i}"] = np.tile(np.asarray(inp[f"b{i}"], np.float32)[None, :],
                                     (128, 1))
    # layer-2 input transform uses raw wl2 on h1 (att folded above), but the
    # xl tables for gathering need att-folded wl2 -- logits use the gathered
    # rows; messages divide by |att| via attrecip (baseline scheme).
    out["w4rep"] = np.tile(np.asarray(inp["w4"], np.float32)[:, 0][None, :], (128, 1))
    out["b4"] = float(np.asarray(inp["b4"], np.float32)[0])
    return out


def _build_x_inputs(x):
    x = np.asarray(x, np.float32)
    ext = np.concatenate([x, np.ones((N, 1), np.float32)], 1)
    n = np.arange(N)
    c, nl = np.divmod(n, NPC)
    b, r = np.divmod(nl, BLK)
    extp = np.zeros((NC * PADN, 5), np.float32)
    extp[c * PADN + b * 128 + r] = ext      # core-major (not table layout)
    xt6_own = np.ascontiguousarray(extp.reshape(NC, PADN, 5).transpose(0, 2, 1))
    return xt6_own


def _build_program(tiles_pb, T, MAXNT):
    import contextlib
    import concourse.bass as bass
    import concourse.bacc as bacc
    import concourse.mybir as mybir
    import concourse.tile as tile

    dt = mybir.dt
    f32 = dt.float32
    bf16 = dt.bfloat16
    i32 = dt.int32
    Alu = mybir.AluOpType
    Act = mybir.ActivationFunctionType
    IOA = bass.IndirectOffsetOnAxis

    nc = bacc.Bacc("TRN2", target_bir_lowering=False, debug=False, num_devices=NC)

    ein = {}
    def EIN(name, shape, d=f32):
        ein[name] = nc.dram_tensor(name, list(shape), d, kind="ExternalInput")
        return ein[name]

    st_blk_d = EIN("st_blk", [NBLK, 128, MAXNT * 128], bf16)
    src_sb_d = EIN("src_sb", [128, T], i32)
    dst_sb_d = EIN("dst_sb", [128, T], bf16)
    extsrc_d = EIN("extsrc", [128, T * 5], bf16)
    xt6_own_d = EIN("xt6_own", [5, PADN])
    wl1arep_d = EIN("wl1arep", [5, NBLK * DIMS[0]], bf16)
    WR1f_d = EIN("WR1f", [5, DIMS[0]])
    WL1B_d = EIN("WL1B", [41, DIMS[0]], bf16)
    WLR2_d = EIN("WLR2", [DIMS[0], 2 * DIMS[1]], bf16)
    WLR3_d = EIN("WLR3", [DIMS[1], 2 * DIMS[2]], bf16)
    weaug_d = [EIN(f"weaug{i}", [1 if i == 1 else 6, NBLK * DIMS[i - 1]], bf16)
               for i in (1, 2, 3)]
    sgnB_d = [EIN(f"sgnB{i}", [128, GBS[i - 1] * DIMS[i - 1]], bf16)
              for i in (1, 2, 3)]
    attrecip_d = [None] + [EIN(f"attrecip{i}", [128, DIMS[i - 1]]) for i in (2, 3)]
    biasRep_d = [EIN(f"biasRep{i}", [128, DIMS[i - 1]]) for i in (1, 2, 3)]
    iota_d = EIN("iota_row", [128, 128], bf16)
    ident_d = EIN("ident", [128, 128], bf16)
    batchloc_d = EIN("batchloc", [128, NBLK], bf16)
    g_rows_d = EIN("g_rows", [128, 1], i32)
    rcnt_d = EIN("rcnt", [128, 4])
    w4rep_d = EIN("w4rep", [128, 8])
    b4_d = EIN("b4v", [128, 1])

    out_d = nc.dram_tensor("out", [B, 1], f32, kind="ExternalOutput")

    stage2 = nc.dram_tensor("stage2", [PADN, DIMS[1]], bf16)
    stage3 = nc.dram_tensor("stage3", [PADN, DIMS[2]], bf16)
    table2 = nc.dram_tensor("table2", [NC * PADN, DIMS[1]], bf16,
                            addr_space="Shared")
    table3 = nc.dram_tensor("table3", [NC * PADN, DIMS[2]], bf16,
                            addr_space="Shared")
    tables = [None, table2, table3]
    pool_full = nc.dram_tensor("pool_full", [POOLPAD, 8], f32)
    pool_red = nc.dram_tensor("pool_red", [B, 8], f32)

    with tile.TileContext(nc) as tc:
        ctx = contextlib.ExitStack()
        with ctx:
            consts = ctx.enter_context(tc.tile_pool(name="consts", bufs=1))
            meta = ctx.enter_context(tc.tile_pool(name="meta", bufs=1))
            xrp = ctx.enter_context(tc.tile_pool(name="xrp", bufs=1))
            stp = ctx.enter_context(tc.tile_pool(name="stp", bufs=2))
            smp = ctx.enter_context(tc.tile_pool(name="smp", bufs=2))
            gp = ctx.enter_context(tc.tile_pool(name="gp", bufs=4))
            sp = ctx.enter_context(tc.tile_pool(name="sp", bufs=3))
            yp = ctx.enter_context(tc.tile_pool(name="yp", bufs=3))
            ep = ctx.enter_context(tc.tile_pool(name="ep", bufs=3))
            pst = ctx.enter_context(tc.tile_pool(name="psum_t", bufs=2, space="PSUM"))
            psb = ctx.enter_context(tc.tile_pool(name="psum_blk", bufs=2, space="PSUM"))
            pse = ctx.enter_context(tc.tile_pool(name="psum_epi", bufs=1, space="PSUM"))
            chp = ctx.enter_context(tc.tile_pool(name="chunk", bufs=2))

            def load_const(dram, shape, d=f32):
                t = consts.tile(list(shape), d, tag=dram.name + "_c")
                nc.sync.dma_start(t[:], dram[:])
                return t
            iota_t = load_const(iota_d, [128, 128], bf16)
            ident_t = load_const(ident_d, [128, 128], bf16)
            WR1f_t = load_const(WR1f_d, [5, DIMS[0]])
            WL1B_t = load_const(WL1B_d, [41, DIMS[0]], bf16)
            WLR2_t = [consts.tile([128, 2 * DIMS[1]], bf16, tag=f"wlr2_{k}",
                                  name=f"wlr2_{k}") for k in range(2)]
            for k in range(2):
                nc.sync.dma_start(WLR2_t[k][:], WLR2_d[k * 128:(k + 1) * 128, :])
            WLR3_t = load_const(WLR3_d, [128, 2 * DIMS[2]], bf16)
            sgnB_t = [load_const(sgnB_d[i], [128, GBS[i] * DIMS[i]], bf16)
                      for i in range(3)]
            attrecip_t = [None] + [load_const(attrecip_d[i - 1], [128, DIMS[i - 1]])
                                   for i in (2, 3)]
            biasRep_t = [load_const(biasRep_d[i], [128, DIMS[i]]) for i in range(3)]
            batchloc_t = load_const(batchloc_d, [128, NBLK], bf16)
            g_rows_t = load_const(g_rows_d, [128, 1], i32)
            rcnt_t = load_const(rcnt_d, [128, 4])
            w4rep_t = load_const(w4rep_d, [128, 8])
            b4_t = load_const(b4_d, [128, 1])
            src_t = meta.tile([128, T], i32)
            nc.sync.dma_start(src_t[:], src_sb_d[:])
            dst_t = meta.tile([128, T], bf16)
            nc.sync.dma_start(dst_t[:], dst_sb_d[:])
            extsrc_t = meta.tile([128, T * 5], bf16)
            nc.sync.dma_start(extsrc_t[:], extsrc_d[:])

            xr_t = [xrp.tile([128, NBLK * DIMS[i]], bf16, tag=f"xr{i}",
                             name=f"xr{i}") for i in range(3)]
            nc.sync.dma_start(xr_t[0][EAROW:EAROW + 1, :], weaug_d[0][:])
            nc.sync.dma_start(xr_t[0][123:128, :], wl1arep_d[:])
            for i in (1, 2):
                nc.sync.dma_start(xr_t[i][EAROW:128, :], weaug_d[i][:])
            # own-node xl tables (self-loop tiles read these instead of gathers)
            xlown_t = [None,
                       xrp.tile([128, NBLK * DIMS[1]], bf16, tag="xlown2",
                                name="xlown2"),
                       xrp.tile([128, NBLK * DIMS[2]], bf16, tag="xlown3",
                                name="xlown3")]

            zero8 = consts.tile([128, 8], f32, tag="zero8")
            nc.gpsimd.memset(zero8[:], 0.0)
            for i in range(POOLPAD // 128):
                nc.sync.dma_start(pool_full[i * 128:(i + 1) * 128, :], zero8[:])

            # ---- preamble: own xr1 (f32 math, bf16 out) ----
            CH = 16
            for ch in range(-(-NBLK // CH)):
                j0, j1 = ch * CH, min((ch + 1) * CH, NBLK)
                xchunk = chp.tile([5, CH * 128], f32, tag="xchunk")
                nc.sync.dma_start(xchunk[:, :(j1 - j0) * 128],
                                  xt6_own_d[:, j0 * 128:j1 * 128])
                for j in range(j1 - j0):
                    b = j0 + j
                    pt = pse.tile([128, DIMS[0]], f32, tag="epi_ps", space="PSUM")
                    nc.tensor.matmul(pt[:], lhsT=xchunk[:, j * 128:(j + 1) * 128],
                                     rhs=WR1f_t[:], start=True, stop=True)
                    D0 = DIMS[0]
                    nc.scalar.copy(xr_t[0][0:BLK, b * D0:(b + 1) * D0],
                                   pt[0:BLK, :])

            # ---- layers ----
            pool_ps = psb.tile([128, 8], f32, tag="pool_ps", space="PSUM", bufs=1)
            for li in range(3):
                H, C = HEADS[li]
                D = DIMS[li]
                D2 = DS[li]
                W = WIDTHS[li]
                GB = GBS[li]
                PSLOT = DIMS[li]
                table = tables[li]
                is_last = li == 2

                if li > 0:
                    stg = stage2 if li == 1 else stage3
                    for k in range(KCH):
                        r0 = int(PB[k]) * 128
                        rn = CHB[k] * 128
                        b0 = int(CHBASE[k])
                        nc.gpsimd.collective_compute(
                            "AllGather", Alu.bypass,
                            replica_groups=[list(range(NC))],
                            ins=[stg[r0:r0 + rn, :].opt()],
                            outs=[table[b0:b0 + NC * rn, :].opt()],
                        )

                t0 = 0
                for b in range(NBLK):
                    nt = tiles_pb[b]
                    pblk = psb.tile([128, W], f32, tag="blk_ps", space="PSUM")
                    sblk = stp.tile([128, MAXNT * 128], bf16, tag="st_blk")
                    nc.scalar.dma_start(sblk[:, :nt * 128],
                                        st_blk_d[b, :, :nt * 128])
                    smat = smp.tile([128, MAXNT * 128], bf16, tag="smat")
                    nc.vector.tensor_tensor(
                        out=smat[:, :nt * 128].rearrange("p (g n) -> p g n", g=nt),
                        in0=dst_t[:, t0:t0 + nt].unsqueeze(2)
                            .to_broadcast([128, nt, 128]),
                        in1=iota_t[:].unsqueeze(1).to_broadcast([128, nt, 128]),
                        op=Alu.is_equal)

                    def do_batch(g0, gs, is_self):
                        ptile = pst.tile([128, GB, PSLOT], f32, tag="t_ps",
                                         space="PSUM")
                        if li > 0 and not is_self:
                            gt = gp.tile([128, GB, D], bf16, tag="g_tile")
                        for i in range(gs):
                            col = g0 + i
                            nc.tensor.matmul(
                                ptile[:, i, 0:D],
                                lhsT=sblk[:, col * 128:(col + 1) * 128],
                                rhs=xr_t[li][:, b * D:(b + 1) * D],
                                start=True, stop=(li == 0))
                            if li == 0:
                                pass
                            elif is_self:
                                nc.tensor.matmul(
                                    ptile[:, i, 0:D], lhsT=ident_t[:],
                                    rhs=xlown_t[li][:, b * D:(b + 1) * D],
                                    start=False, stop=True)
                            else:
                                t = t0 + col
                                nc.gpsimd.indirect_dma_start(
                                    out=gt[:, i, :], out_offset=None,
                                    in_=table[:],
                                    in_offset=IOA(ap=src_t[:, t:t + 1], axis=0))
                                nc.tensor.matmul(
                                    ptile[:, i, 0:D], lhsT=ident_t[:],
                                    rhs=gt[:, i, :], start=False, stop=True)
                        u = sp.tile([128, GBS[0] * DIMS[0]], bf16, tag="u_t")
                        nc.scalar.activation(
                            u[:, :gs * D].rearrange("p (g d) -> p g d", g=gs),
                            ptile[:, 0:gs, 0:D], Act.Prelu, alpha=0.2)
                        v = sp.tile([128, GBS[0] * DIMS[0]], bf16, tag="v_t")
                        nc.vector.tensor_tensor(
                            out=v[:, :gs * D], in0=u[:, :gs * D],
                            in1=sgnB_t[li][:, :gs * D], op=Alu.mult)
                        lg = sp.tile([128, GBS[1] * 8], f32, tag="lg")
                        nc.vector.tensor_reduce(
                            out=lg[:, :gs * H].rearrange("p (g h) -> p g h", g=gs),
                            in_=v[:, :gs * D].rearrange("p (g h c) -> p g h c",
                                                        g=gs, h=H),
                            axis=mybir.AxisListType.X, op=Alu.add)
                        yt = yp.tile([128, GB, W], bf16, tag="y_tile")
                        nc.scalar.activation(
                            yt[:, 0:gs, D2:W],
                            lg[:, :gs * H].rearrange("p (g h) -> p g h", g=gs),
                            Act.Exp)
                        if li == 0:
                            nc.vector.tensor_tensor(
                                out=yt[:, 0:gs, 0:D2]
                                    .rearrange("p g (h k) -> p g h k", h=H),
                                in0=extsrc_t[:, (t0 + g0) * 5:(t0 + g0 + gs) * 5]
                                    .rearrange("p (g k) -> p g k", g=gs)
                                    .unsqueeze(2).to_broadcast([128, gs, H, 5]),
                                in1=yt[:, 0:gs, D2:W].unsqueeze(3)
                                    .to_broadcast([128, gs, H, 5]),
                                op=Alu.mult)
                        else:
                            if is_self:
                                in0 = xlown_t[li][:, b * D:(b + 1) * D] \
                                    .rearrange("p (h c) -> p h c", h=H) \
                                    .unsqueeze(1)
                            else:
                                in0 = gt[:, 0:gs, :].rearrange(
                                    "p g (h c) -> p g h c", h=H)
                            nc.vector.tensor_tensor(
                                out=yt[:, 0:gs, 0:D2]
                                    .rearrange("p g (h c) -> p g h c", h=H),
                                in0=in0,
                                in1=yt[:, 0:gs, D2:W].unsqueeze(3)
                                    .to_broadcast([128, gs, H, C]),
                                op=Alu.mult)
                        for i in range(gs):
                            col = g0 + i
                            nc.tensor.matmul(
                                pblk[:], lhsT=smat[:, col * 128:(col + 1) * 128],
                                rhs=yt[:, i, 0:W],
                                start=(col == 0), stop=(col == nt - 1))

                    if li == 0:
                        for g0 in range(0, nt, GB):
                            do_batch(g0, min(GB, nt - g0), False)
                    else:
                        for g0 in range(0, nt - 1, GB):
                            do_batch(g0, min(GB, nt - 1 - g0), False)
                        do_batch(nt - 1, 1, True)
                    t0 += nt

                    # ---- block epilogue ----
                    den = sp.tile([128, 8], f32, tag="den")
                    nc.vector.tensor_scalar_add(den[:, 0:H], pblk[:, D2:W], 1e-30)
                    rden = sp.tile([128, 8], f32, tag="rden")
                    nc.vector.reciprocal(rden[:, 0:H], den[:, 0:H])
                    if li == 0:
                        hr5 = ep.tile([128, 41], bf16, tag="hr5")
                        nc.vector.memset(hr5[:, 40:41], 1.0)
                        nc.vector.tensor_tensor(
                            out=hr5[:, 0:40].rearrange("p (h k) -> p h k", h=8),
                            in0=pblk[:, 0:40].rearrange("p (h k) -> p h k", h=8),
                            in1=rden[:, 0:8].unsqueeze(2)
                                .to_broadcast([128, 8, 5]),
                            op=Alu.mult)
                        tp = pse.tile([41, 128], bf16, tag="epi_ps", space="PSUM")
                        nc.tensor.transpose(tp[:], hr5[:], ident_t[:])
                        hT5 = ep.tile([41, 128], bf16, tag="hT5")
                        nc.scalar.copy(hT5[:], tp[:])
                        ph = pse.tile([128, DIMS[0]], f32, tag="epi_ps",
                                      space="PSUM")
                        nc.tensor.matmul(ph[:], lhsT=hT5[:], rhs=WL1B_t[:],
                                         start=True, stop=True)
                        h = ep.tile([128, DIMS[0]], bf16, tag="h_blk")
                        nc.scalar.activation(h[:], ph[:], Act.Tanh)
                    else:
                        hr = ep.tile([128, DIMS[0]], f32, tag="hr")
                        nc.vector.tensor_tensor(
                            out=hr[:, 0:D].rearrange("p (h c) -> p h c", h=H),
                            in0=pblk[:, 0:D].rearrange("p (h c) -> p h c", h=H),
                            in1=rden[:, 0:H].unsqueeze(2)
                                .to_broadcast([128, H, C]),
                            op=Alu.mult)
                        nc.vector.tensor_tensor(out=hr[:, 0:D], in0=hr[:, 0:D],
                                                in1=attrecip_t[li][:], op=Alu.mult)
                        nc.vector.tensor_tensor(out=hr[:, 0:D], in0=hr[:, 0:D],
                                                in1=biasRep_t[li][:], op=Alu.add)
                        h = ep.tile([128, DIMS[0]], bf16, tag="h_blk")
                        nc.scalar.activation(h[:, 0:D], hr[:, 0:D], Act.Tanh)

                    if not is_last:
                        D2n = DIMS[li + 1]
                        WLRn = [WLR2_t[0], WLR2_t[1]] if li == 0 else [WLR3_t]
                        nk = DIMS[li] // 128
                        hT = []
                        for k in range(nk):
                            tpk = pse.tile([128, 128], bf16, tag="epi_ps",
                                           space="PSUM")
                            nc.tensor.transpose(tpk[:], h[:, k * 128:(k + 1) * 128],
                                                ident_t[:])
                            hTk = ep.tile([128, 128], bf16, tag=f"hT{k}")
                            nc.scalar.copy(hTk[:], tpk[:])
                            hT.append(hTk)
                        pxlr = pse.tile([128, 2 * D2n], f32, tag="epi_ps",
                                        space="PSUM")
                        for k in range(nk):
                            nc.tensor.matmul(pxlr[:], lhsT=hT[k][:],
                                             rhs=WLRn[k][:],
                                             start=(k == 0), stop=(k == nk - 1))
                        xlo = xlown_t[li + 1]
                        nc.scalar.copy(xlo[:, b * D2n:(b + 1) * D2n],
                                       pxlr[:, 0:D2n])
                        nc.sync.dma_start(
                            (stage2 if li == 0 else stage3)[b * 128:(b + 1) * 128, :],
                            xlo[:, b * D2n:(b + 1) * D2n])
                        nc.scalar.copy(
                            xr_t[li + 1][0:BLK, b * D2n:(b + 1) * D2n],
                            pxlr[0:BLK, D2n:2 * D2n])
                    else:
                        Sg = smp.tile([128, 128], bf16, tag="sg_tile")
                        nc.vector.tensor_tensor(
                            out=Sg[:],
                            in0=batchloc_t[:, b:b + 1].to_broadcast([128, 128]),
                            in1=iota_t[:], op=Alu.is_equal)
                        nc.tensor.matmul(pool_ps[:], lhsT=Sg[:], rhs=h[:, 0:8],
                                         start=(b == 0), stop=(b == NBLK - 1))

            # ---- pooling + head ----
            pool_sb = ep.tile([128, 8], f32, tag="pool_sb")
            nc.vector.tensor_copy(pool_sb[:], pool_ps[:])
            nc.gpsimd.indirect_dma_start(
                out=pool_full[:], out_offset=IOA(ap=g_rows_t[:, :1], axis=0),
                in_=pool_sb[:], in_offset=None)
            nc.gpsimd.collective_compute(
                "AllReduce", Alu.add, replica_groups=[list(range(NC))],
                ins=[pool_full.ap()[0:B, :].opt()], outs=[pool_red.ap().opt()])
            for i in range(B // 128):
                pt = ep.tile([128, 8], f32, tag="head_in")
                nc.sync.dma_start(pt[:], pool_red[i * 128:(i + 1) * 128, :])
                pw = ep.tile([128, 8], f32, tag="head_w")
                nc.vector.tensor_tensor(out=pw[:], in0=pt[:], in1=w4rep_t[:],
                                        op=Alu.mult)
                hred = ep.tile([128, 1], f32, tag="head_red")
                nc.vector.tensor_reduce(out=hred[:], in_=pw[:],
                                        axis=mybir.AxisListType.X, op=Alu.add)
                nc.vector.tensor_tensor(out=hred[:], in0=hred[:],
                                        in1=rcnt_t[:, i:i + 1], op=Alu.mult)
                nc.vector.tensor_tensor(out=hred[:], in0=hred[:], in1=b4_t[:],
                                        op=Alu.add)
                nc.sync.dma_start(out_d[i * 128:(i + 1) * 128, :], hred[:])

    nc.compile()
    return nc


def _get_program(inputs):
    pre = _host_preprocess(inputs["x"], inputs["edge_index"], inputs["edge_attr"],
                           inputs["batch"])
    key = tuple(pre["tiles_pb"])
    if key not in _CACHE:
        _CACHE[key] = _build_program(pre["tiles_pb"], pre["T"], pre["MAXNT"])
    return _CACHE[key], pre


def _make_in_maps(inputs, pre):
    import ml_dtypes
    bf16 = ml_dtypes.bfloat16
    wts = _host_weights(inputs)
    xt6_own = _build_x_inputs(inputs["x"])
    iota = np.tile(np.arange(128, dtype=np.float32), (128, 1))
    ident = np.eye(128, dtype=np.float32)
    in_maps = []
    for c in range(NC):
        m = dict(
            st_blk=pre["st_blk"][c].astype(bf16), src_sb=pre["src_sb"][c],
            dst_sb=pre["dst_sb"][c].astype(bf16),
            extsrc=pre["extsrc"][c].astype(bf16),
            xt6_own=xt6_own[c],
            wl1arep=wts["wl1arep"].astype(bf16), WR1f=wts["WR1f"],
            WL1B=wts["WL1B"].astype(bf16),
            WLR2=wts["WLR2"].astype(bf16), WLR3=wts["WLR3"].astype(bf16),
            iota_row=iota.astype(bf16), ident=ident.astype(bf16),
            batchloc=pre["batchloc"][c].astype(bf16), g_rows=pre["g_rows"][c],
            rcnt=np.ascontiguousarray(pre["rcnt"].reshape(4, 128).T),
            w4rep=wts["w4rep"], b4v=np.full((128, 1), wts["b4"], np.float32),
        )
        for i in (1, 2, 3):
            m[f"weaug{i}"] = wts[f"weaug{i}"].astype(bf16)
            m[f"sgnB{i}"] = wts[f"sgnB{i}"].astype(bf16)
            m[f"biasRep{i}"] = wts[f"biasRep{i}"]
            if i > 1:
                m[f"attrecip{i}"] = wts[f"attrecip{i}"]
        in_maps.append(m)
    return in_maps


def kernel(**inputs):
    from concourse.bass_utils import run_bass_kernel_spmd
    nc, pre = _get_program(inputs)
    in_maps = _make_in_maps(inputs, pre)
    res = run_bass_kernel_spmd(nc, in_maps, core_ids=list(range(NC)))
    return np.asarray(res.results[0]["out"], np.float32)
